# revision 33
# baseline (speedup 1.0000x reference)
"""LOCA-style kernel for Trainium2, data-parallel over batch on 8 NeuronCores.

Per core (one batch element), per step:
  - depthwise 3x3 correlation for D0=conv(w0-w2), D1=conv(w1-w2): 7 taps
    as fp16 diagonal-weight PE matmuls on a flat-raster feature map
    (zero-edge-column variants give exact padding); the 2 remaining (dy,0)
    taps are fused with the PSUM merge via DVE affine_then_add
    (out = f_shift*wdy + acc), one op per tap per granule.
  - R2's head contribution is linear, so it never materializes per-channel:
    P[dx, n] = sum_dy sum_c (w_head*w2)[c,dy,dx] * f[c, n+64*dy] via 6
    matmul passes with [128 -> {0,32,64}] stationaries, then 3 tiny K=1
    shift-matmuls per 512-chunk accumulate P's dx-shifts straight into the
    head PSUM (edge wrap killed by strided memsets on the SBUF copy).
  - softmax-weighted object sum via the shift identity
      red = R2 + (D0*e0 + D1*e1) / (1 + e0 + e1),  e_i = exp(D_i)
    with exps/Ln on ScalarE (|w_head| folded into the Ln scale/bias),
    products and sums on DVE fp16 2x mode, all in-place on 4 full-map tags.
  - 1x1 head with sign(w_head) stationary + ReLU + 8x bilinear upsample
    as two separable matmul passes. Output fp16, upcast on host.
"""

import sys

sys.path.insert(0, "/opt/trn_rl_repo")

import numpy as np
from contextlib import ExitStack

import concourse.bass as bass
import concourse.mybir as mybir
from concourse import bacc, tile
from concourse.bass_utils import run_bass_kernel_spmd

BS, C, H, W = 8, 256, 64, 64
STEPS, NO = 3, 3
RED = 8
HO, WO = H * RED, W * RED  # 512, 512
NCORES = 8
NCT = 2
HW = H * W  # 4096
GR = 1024  # conv psum granule (pixels)
NGR = HW // GR  # 4
F16 = mybir.dt.float16
F32 = mybir.dt.float32
AF = mybir.ActivationFunctionType
ALU = mybir.AluOpType

# flat f8 layout: one tile [128, 3*BLK] per ct;
#   block 0 (Vz63): col x=63 zeroed, x-origin at 65 (for dx=-1 taps)
#   block 1 (Vz0):  col x=0 zeroed, x-origin at 65 (for dx=+1 taps)
#   block 2 (V0):   full map, x-origin at 64 (for dx=0 taps)
BLK = 4232  # 64 head pad + 4096 + 72 tail pad, even
B1, B2, B3 = 0, BLK, 2 * BLK
FTOT = 3 * BLK
FSPLIT = 2180  # first-piece DMA covers granules 0-1 of every tap


def _tap_off(dy, dx):
    if dx == -1:
        return B1 + 64 + 64 * dy
    if dx == 1:
        return B2 + 66 + 64 * dy
    return B3 + 64 + 64 * dy


# PE taps (x-shifted + center, fp16 diag matmuls); off-PE taps ride DVE
# affine_then_add ops that fuse tap multiply with the psum merge.
PTAPS = [(-1, -1), (-1, 1), (0, -1), (0, 0), (0, 1), (1, -1), (1, 1)]
OTAPS = [(-1, 0), (1, 0)]
NPT = len(PTAPS)
PTAP_OFFS = [_tap_off(*t) for t in PTAPS]
for _o in PTAP_OFFS:
    assert _o % 2 == 0, _o
OTAP_OFFS = [_tap_off(*t) for t in OTAPS]

PQW = 1 + HW + 1  # padded Pq row width


def _bilinear_matrix(n_in: int, n_out: int) -> np.ndarray:
    U = np.zeros((n_out, n_in), np.float64)
    s = n_in / n_out
    for i in range(n_out):
        c = (i + 0.5) * s - 0.5
        lo = int(np.floor(c))
        f = c - lo
        for idx, wt in ((lo, 1.0 - f), (lo + 1, f)):
            U[i, min(max(idx, 0), n_in - 1)] += wt
    return U


def _host_prep(f_e, all_prototypes, w_head, b_head):
    f_e = np.asarray(f_e, np.float32)
    ap = np.asarray(all_prototypes, np.float32)
    w_head = np.asarray(w_head, np.float32)
    b_val = float(np.asarray(b_head).reshape(-1)[0])

    # ---- flat fp16 variants ----
    f16 = f_e.astype(np.float16)  # [BS, C, H, W]
    z63 = f16.copy()
    z63[:, :, :, 63] = 0
    z0 = f16.copy()
    z0[:, :, :, 0] = 0
    fblk = np.zeros((BS, NCT, 128, FTOT), np.float16)
    for ct in range(NCT):
        sl = slice(ct * 128, (ct + 1) * 128)
        fblk[:, ct, :, B1 + 65: B1 + 65 + HW] = z63[:, sl].reshape(BS, 128, HW)
        fblk[:, ct, :, B2 + 65: B2 + 65 + HW] = z0[:, sl].reshape(BS, 128, HW)
        fblk[:, ct, :, B3 + 64: B3 + 64 + HW] = f16[:, sl].reshape(BS, 128, HW)

    # ---- conv weights: D0 = w0-w2, D1 = w1-w2 (fp16) ----
    # ap[s, o*9+t, b, c] -> wm[b, s, o, t(9), c]
    wm = ap.transpose(2, 0, 1, 3).reshape(BS, STEPS, NO, 9, C)
    v = np.stack([wm[:, :, 0] - wm[:, :, 2], wm[:, :, 1] - wm[:, :, 2]], axis=2)
    vf = v.astype(np.float16).astype(np.float32)  # [BS, S, 2, 9, C]

    tapidx = lambda dy, dx: (dy + 1) * 3 + (dx + 1)

    # PE-tap diag stationaries [BS, S, NCT, 2conv, 7tap, 128, 128] fp16
    diag = np.zeros((BS, STEPS, NCT, 2, NPT, 128, 128), np.float16)
    cidx = np.arange(128)
    for ct in range(NCT):
        sl = slice(ct * 128, (ct + 1) * 128)
        for pi, tp in enumerate(PTAPS):
            wa = vf[:, :, :, tapidx(*tp), sl]  # [BS, S, 2, 128]
            diag[:, :, ct, :, pi, cidx, cidx] = wa.transpose(3, 0, 1, 2).astype(np.float16)
    # reorder for per-(step,ct) DMA: -> [BS, S, NCT, 128, 2, 7, 128]
    diag = np.ascontiguousarray(diag.transpose(0, 1, 2, 5, 3, 4, 6))

    # fused-tap scalars for OTAPS: [BS, S, 128, NCT*2conv*3tap] f32
    NOT = len(OTAPS)
    wdy = np.zeros((BS, STEPS, 128, NCT * 2 * NOT), np.float32)
    for ct in range(NCT):
        sl = slice(ct * 128, (ct + 1) * 128)
        for cv in range(2):
            for i, (dy, dx) in enumerate(OTAPS):
                wdy[:, :, :, (ct * 2 + cv) * NOT + i] = vf[:, :, cv, tapidx(dy, dx), sl]

    # R2-head stationaries: u = w_head * w2; ug[b, s, ct, dy, c128, 65] fp16
    # (columns 0/32/64 hold the dx=-1/0/+1 weights so P rows land on
    #  partitions 0/32/64, which are legal K-tile bases for the shift matmuls)
    u = wm[:, :, 2] * w_head[None, None, None, :]  # [BS, S, 9tap, C]
    ug = np.zeros((BS, STEPS, NCT, 3, 128, 65), np.float16)
    for ct in range(NCT):
        sl = slice(ct * 128, (ct + 1) * 128)
        for dyi in range(3):
            for dxi in range(3):
                ug[:, :, ct, dyi, :, 32 * dxi] = u[:, :, dyi * 3 + dxi, sl].astype(np.float16)

    absw = np.abs(w_head).astype(np.float64)
    invw = np.where(absw > 0, 1.0 / np.maximum(absw, 1e-30), 1.0e30)
    invw = np.minimum(invw, 1.0e30).astype(np.float32)
    signw = np.sign(w_head).astype(np.float16)
    invw_t = np.ascontiguousarray(invw.reshape(NCT, 128, 1))
    signw_t = np.ascontiguousarray(signw.reshape(NCT, 128, 1))

    ut = _bilinear_matrix(H, HO).T.astype(np.float16)  # [64, 512]
    eye = np.eye(128, dtype=np.float16)
    ones3 = np.zeros((65, 1), np.float16)
    ones3[[0, 32, 64], 0] = 1.0

    in_maps = []
    for b in range(BS):
        in_maps.append(
            {
                "fblk": np.ascontiguousarray(fblk[b]),
                "diag": np.ascontiguousarray(diag[b]),
                "wdy": np.ascontiguousarray(wdy[b]),
                "ug": np.ascontiguousarray(ug[b]),
                "invw": invw_t,
                "signw": signw_t,
                "ut": ut,
                "eye": eye,
                "ones3": ones3,
            }
        )
    return in_maps, b_val


def _build_nc(b_val: float) -> bass.Bass:
    nc = bacc.Bacc(None, target_bir_lowering=False)
    fblk_d = nc.declare_dram_parameter("fblk", [NCT, 128, FTOT], F16, isOutput=False)
    diag_d = nc.declare_dram_parameter("diag", [STEPS, NCT, 128, 2 * NPT * 128], F16, isOutput=False)
    wdy_d = nc.declare_dram_parameter("wdy", [STEPS, 128, NCT * 2 * 2], F32, isOutput=False)
    ug_d = nc.declare_dram_parameter("ug", [STEPS, NCT, 3, 128, 65], F16, isOutput=False)
    invw_d = nc.declare_dram_parameter("invw", [NCT, 128, 1], F32, isOutput=False)
    signw_d = nc.declare_dram_parameter("signw", [NCT, 128, 1], F16, isOutput=False)
    ut_d = nc.declare_dram_parameter("ut", [64, WO], F16, isOutput=False)
    eye_d = nc.declare_dram_parameter("eye", [128, 128], F16, isOutput=False)
    ones3_d = nc.declare_dram_parameter("ones3", [65, 1], F16, isOutput=False)
    out_d = nc.declare_dram_parameter("out", [STEPS, HO, WO], F16, isOutput=True)

    with tile.TileContext(nc) as tc, ExitStack() as ctx:
        const = ctx.enter_context(tc.tile_pool(name="const", bufs=1))
        fpool = ctx.enter_context(tc.tile_pool(name="fpool", bufs=1))
        dpool = ctx.enter_context(tc.tile_pool(name="dpool", bufs=2))
        upool = ctx.enter_context(tc.tile_pool(name="upool", bufs=2))
        vpool = ctx.enter_context(tc.tile_pool(name="vpool", bufs=2))
        qpool = ctx.enter_context(tc.tile_pool(name="qpool", bufs=1))
        opool = ctx.enter_context(tc.tile_pool(name="opool", bufs=2))
        ps_d = ctx.enter_context(tc.tile_pool(name="ps_d", bufs=3, space="PSUM"))
        ps_p = ctx.enter_context(tc.tile_pool(name="ps_p", bufs=1, space="PSUM"))
        ps_head = ctx.enter_context(tc.tile_pool(name="ps_head", bufs=1, space="PSUM"))

        # ---- first-needed data first: step-0/ct-0 weights, then features ----
        dg00 = dpool.tile([128, 2 * NPT * 128], F16, tag="diag")
        nc.sync.dma_start(out=dg00[:], in_=diag_d[0, 0])
        fsb = []
        for ct in range(NCT):
            t = fpool.tile([128, FTOT], F16, tag=f"f{ct}")
            fsb.append(t)
        # staged feature DMA on the (otherwise idle) GpSimd DMA queue so the
        # step-0 weight DMAs on the sync queue never wait behind it
        def _fpiece(ct, lo, hi):
            nc.gpsimd.dma_start(
                out=fsb[ct][:].rearrange("p (b x) -> p b x", b=3)[:, :, lo:hi],
                in_=fblk_d[ct].rearrange("p (b x) -> p b x", b=3)[:, :, lo:hi],
            )

        _fpiece(0, 0, 1160)
        # prefetch the rest of step 0's weights before the bulk features
        dg01 = dpool.tile([128, 2 * NPT * 128], F16, tag="diag")
        nc.sync.dma_start(out=dg01[:], in_=diag_d[0, 1])
        ug00 = []
        for dyi in range(3):
            t = dpool.tile([128, 65], F16, tag=f"ug{dyi}")
            nc.sync.dma_start(out=t[:], in_=ug_d[0, 0, dyi])
            ug00.append(t)
        _fpiece(0, 1160, FSPLIT)
        _fpiece(1, 0, FSPLIT)
        _fpiece(0, FSPLIT, BLK)
        _fpiece(1, FSPLIT, BLK)

        # ---- constants ----
        ut_sb = const.tile([64, WO], F16, tag="ut")
        nc.sync.dma_start(out=ut_sb[:], in_=ut_d[:])
        eye_sb = const.tile([128, 128], F16, tag="eye")
        nc.sync.dma_start(out=eye_sb[:], in_=eye_d[:])
        ones3_sb = const.tile([65, 1], F16, tag="ones3")
        nc.sync.dma_start(out=ones3_sb[:], in_=ones3_d[:])
        invw_sb, signw_sb = [], []
        for ct in range(NCT):
            t = const.tile([128, 1], F32, tag=f"invw{ct}")
            nc.sync.dma_start(out=t[:], in_=invw_d[ct])
            invw_sb.append(t)
            t = const.tile([128, 1], F16, tag=f"signw{ct}")
            nc.sync.dma_start(out=t[:], in_=signw_d[ct])
            signw_sb.append(t)

        def emit_head(s, rsp_ct, pq):
            # ---- head: dmap chunks = R2h shifts + sign(w)-weighted t sums ----
            dmY = opool.tile([64, 64], F16, tag="dmY")
            for k in range(HW // 512):
                pd = ps_head.tile([1, 512], F32, tag="hps")
                for j in range(3):
                    base = 32 * j
                    nc.tensor.matmul(
                        pd[:],
                        eye_sb[0:65, base: base + 1],
                        pq[0:65, j + k * 512: j + k * 512 + 512],
                        start=(j == 0),
                        stop=False,
                    )
                for ct in range(NCT):
                    nc.tensor.matmul(
                        pd[:],
                        signw_sb[ct][:],
                        rsp_ct[ct][0][:, k * 512: (k + 1) * 512],
                        start=False,
                        stop=(ct == NCT - 1),
                    )
                dm1 = opool.tile([1, 512], F16, tag="dm1")
                nc.scalar.activation(dm1[:], pd[:], AF.Relu, bias=b_val)
                nc.sync.dma_start(
                    out=dmY[8 * k: 8 * k + 8, :],
                    in_=dm1[:].rearrange("p (y x) -> p y x", x=64),
                )
            # transpose -> x on partitions
            psT0 = ps_head.tile([64, 64], F16, tag="hps")
            nc.tensor.transpose(psT0[:], dmY[:], eye_sb[0:64, 0:64])
            dmX = opool.tile([64, 64], F16, tag="dmX")
            nc.vector.tensor_copy(dmX[:], psT0[:])

            # horizontal upsample in one matmul: H[y, X] (dmX stationary)
            ps_h = ps_head.tile([64, 512], F32, tag="hps")
            nc.tensor.matmul(ps_h[:], dmX[:], ut_sb[:], start=True, stop=True)
            h_sb = opool.tile([64, 512], F16, tag="h_sb")
            nc.scalar.activation(h_sb[:], ps_h[:], AF.Copy)
            # vertical upsample: out[Y, X] = sum_y ut[y, Y] * H[y, X]
            for yc in range(4):
                pv = ps_head.tile([128, 512], F32, tag="hps")
                nc.tensor.matmul(
                    pv[:],
                    ut_sb[:, yc * 128: (yc + 1) * 128],
                    h_sb[:],
                    start=True,
                    stop=True,
                )
                osb = opool.tile([128, 512], F16, tag="osb")
                nc.scalar.activation(osb[:], pv[:], AF.Copy)
                nc.sync.dma_start(out=out_d[s, yc * 128: (yc + 1) * 128, :], in_=osb[:])

        for s in range(STEPS):
            wy = dpool.tile([128, NCT * 2 * 2], F32, tag="wdy")
            nc.sync.dma_start(out=wy[:], in_=wdy_d[s])

            # R2-head stationaries for this step
            ug_sb = []
            for ct in range(NCT):
                if s == 0 and ct == 0:
                    ug_sb.append(ug00)
                else:
                    tiles = []
                    for dyi in range(3):
                        t = dpool.tile([128, 65], F16, tag=f"ug{ct}_{dyi}" if ct else f"ug{dyi}")
                        nc.sync.dma_start(out=t[:], in_=ug_d[s, ct, dyi])
                        tiles.append(t)
                    ug_sb.append(tiles)

            # ---- convs (PE taps + fused DVE taps/merge) + R2h P-pass ----
            rsp_ct = []
            pq = qpool.tile([65, PQW], F16, tag="pq")
            for ct in range(NCT):
                if s == 0 and ct == 0:
                    dg = dg00
                elif s == 0 and ct == 1:
                    dg = dg01
                else:
                    dg = dpool.tile([128, 2 * NPT * 128], F16, tag="diag")
                    nc.sync.dma_start(out=dg[:], in_=diag_d[s, ct])
                dgv = dg[:].rearrange("p (c pt x) -> p c pt x", c=2, pt=NPT)
                rsp = []
                for cv in range(2):
                    r = vpool.tile([128, HW + 32 * cv], F16, tag=f"rsp{ct}{cv}")
                    rsp.append(r)
                rsp_ct.append(rsp)
                for g in range(NGR):
                    gsl = slice(g * GR, (g + 1) * GR)
                    for cv in range(2):
                        acc = ps_d.tile([128, GR], F32, tag="acc")
                        for pi in range(NPT):
                            stat = dgv[:, cv, pi, :]
                            off = PTAP_OFFS[pi] + g * GR
                            for sub in range(GR // 512):
                                nc.tensor.matmul(
                                    acc[:, sub * 512: (sub + 1) * 512],
                                    stat,
                                    fsb[ct][:, off + sub * 512: off + (sub + 1) * 512],
                                    start=(pi == 0),
                                    stop=(pi == NPT - 1),
                                )
                        # fused off-PE taps + psum merge on DVE
                        wix = (ct * 2 + cv) * 2
                        r = rsp[cv]
                        nc.vector.affine_then_add(
                            out=r[:, gsl],
                            in0=fsb[ct][:, OTAP_OFFS[0] + g * GR: OTAP_OFFS[0] + (g + 1) * GR],
                            in1=acc[:],
                            scale=wy[:, wix: wix + 1],
                            bias=0.0,
                        )
                        nc.vector.affine_then_add(
                            out=r[:, gsl],
                            in0=fsb[ct][:, OTAP_OFFS[1] + g * GR: OTAP_OFFS[1] + (g + 1) * GR],
                            in1=r[:, gsl],
                            scale=wy[:, wix + 1: wix + 2],
                            bias=0.0,
                        )
            # P psum chunks: [65, 512] x8, 6 matmuls each (3 dy x 2 ct)
            for k in range(HW // 512):
                pp = ps_p.tile([65, 512], F32, tag="pp")
                first = True
                for ct in range(NCT):
                    for dyi, dy in enumerate((-1, 0, 1)):
                        off = B3 + 64 + 64 * dy + k * 512
                        nc.tensor.matmul(
                            pp[:],
                            ug_sb[ct][dyi][:],
                            fsb[ct][:, off: off + 512],
                            start=first,
                            stop=(ct == NCT - 1 and dyi == 2),
                        )
                        first = False
                nc.scalar.activation(pq[:, 1 + k * 512: 1 + (k + 1) * 512], pp[:], AF.Copy)
            # edge fixes on Pq: zero pads (all rows: avoid NaN*0 from garbage)
            nc.vector.memset(pq[0:65, 0:1], 0.0)
            nc.vector.memset(pq[0:65, PQW - 1: PQW], 0.0)
            pqv0 = pq[0:1, 1: 1 + HW].rearrange("p (y x) -> p y x", x=64)
            nc.vector.memset(pqv0[:, :, 63:64], 0.0)
            pqv2 = pq[64:65, 1: 1 + HW].rearrange("p (y x) -> p y x", x=64)
            nc.vector.memset(pqv2[:, :, 0:1], 0.0)

            # ---- softmax chain (in place, halves) ----
            e_ct = []
            hsls = [slice(0, HW // 2), slice(HW // 2, HW)]
            for ct in range(NCT):
                e0 = upool.tile([128, HW + 32], F16, tag=f"e0{ct}")
                e1 = upool.tile([128, HW + 64], F16, tag=f"e1{ct}")
                e_ct.append((e0, e1))
                for hs in hsls:
                    nc.scalar.activation(e0[:, hs], rsp_ct[ct][0][:, hs], AF.Exp)
                    nc.scalar.activation(e1[:, hs], rsp_ct[ct][1][:, hs], AF.Exp)
            for ct in range(NCT):
                D0r, D1r = rsp_ct[ct]
                e0, e1 = e_ct[ct]
                for hs in hsls:
                    nc.vector.tensor_mul(D0r[:, hs], D0r[:, hs], e0[:, hs])
                    nc.vector.tensor_mul(D1r[:, hs], D1r[:, hs], e1[:, hs])
                    nc.vector.tensor_add(D0r[:, hs], D0r[:, hs], D1r[:, hs])
                    nc.vector.tensor_add(e0[:, hs], e0[:, hs], e1[:, hs])
            for ct in range(NCT):
                e0, e1 = e_ct[ct]
                nc.scalar.activation(e1[:, 0:HW], e0[:, 0:HW], AF.Ln, scale=invw_sb[ct][:, 0:1], bias=invw_sb[ct][:, 0:1])
            for ct in range(NCT):
                e0, e1 = e_ct[ct]
                nc.scalar.activation(e0[:, 0:HW], e1[:, 0:HW], AF.Exp, scale=-1.0)
            for ct in range(NCT):
                D0r, D1r = rsp_ct[ct]
                e0, e1 = e_ct[ct]
                for hs in hsls:
                    nc.vector.tensor_mul(D0r[:, hs], D0r[:, hs], e0[:, hs])

            emit_head(s, rsp_ct, pq)

    nc.compile()
    return nc


_CACHE = {}


def _get_nc(b_val: float) -> bass.Bass:
    key = round(b_val, 12)
    if key not in _CACHE:
        _CACHE[key] = _build_nc(b_val)
    return _CACHE[key]


def kernel(f_e, all_prototypes, w_head, b_head):
    in_maps, b_val = _host_prep(f_e, all_prototypes, w_head, b_head)
    nc = _get_nc(b_val)
    res = run_bass_kernel_spmd(nc, in_maps, list(range(NCORES)), trace=False)
    outs = [res.results[b]["out"].reshape(STEPS, 1, HO, WO) for b in range(BS)]
    full = np.stack(outs, axis=1)  # [STEPS, BS, 1, HO, WO]
    return full.astype(np.float32)


# revision 34
# speedup vs baseline: 1.0759x; 1.0759x over previous
"""LOCA-style kernel for Trainium2, data-parallel over batch on 8 NeuronCores.

Per core (one batch element), per step:
  - depthwise 3x3 correlation for D0=conv(w0-w2), D1=conv(w1-w2): 7 taps
    as fp16 diagonal-weight PE matmuls on a flat-raster feature map
    (zero-edge-column variants give exact padding); the 2 remaining (dy,0)
    taps are fused with the PSUM merge via DVE affine_then_add
    (out = f_shift*wdy + acc), one op per tap per granule.
  - R2's head contribution is linear, so it never materializes per-channel:
    P[dx, n] = sum_dy sum_c (w_head*w2)[c,dy,dx] * f[c, n+64*dy] via 6
    matmul passes with [128 -> {0,32,64}] stationaries, then 3 tiny K=1
    shift-matmuls per 512-chunk accumulate P's dx-shifts straight into the
    head PSUM (edge wrap killed by strided memsets on the SBUF copy).
  - softmax-weighted object sum via the shift identity
      red = R2 + (D0*e0 + D1*e1) / (1 + e0 + e1),  e_i = exp(D_i)
    with exps/Ln on ScalarE (|w_head| folded into the Ln scale/bias),
    products and sums on DVE fp16 2x mode, all in-place on 4 full-map tags.
  - 1x1 head with sign(w_head) stationary + ReLU + 8x bilinear upsample
    as two separable matmul passes. Output fp16, upcast on host.
"""

import sys

sys.path.insert(0, "/opt/trn_rl_repo")

import numpy as np
from contextlib import ExitStack

import concourse.bass as bass
import concourse.mybir as mybir
from concourse import bacc, tile
from concourse.bass_utils import run_bass_kernel_spmd

BS, C, H, W = 8, 256, 64, 64
STEPS, NO = 3, 3
RED = 8
HO, WO = H * RED, W * RED  # 512, 512
NCORES = 8
NCT = 2
HW = H * W  # 4096
GR = 1024  # conv psum granule (pixels)
NGR = HW // GR  # 4
F16 = mybir.dt.float16
F32 = mybir.dt.float32
AF = mybir.ActivationFunctionType
ALU = mybir.AluOpType

# flat f8 layout: one tile [128, 3*BLK] per ct;
#   block 0 (Vz63): col x=63 zeroed, x-origin at 65 (for dx=-1 taps)
#   block 1 (Vz0):  col x=0 zeroed, x-origin at 65 (for dx=+1 taps)
#   block 2 (V0):   full map, x-origin at 64 (for dx=0 taps)
BLK = 4232  # 64 head pad + 4096 + 72 tail pad, even
B1, B2, B3 = 0, BLK, 2 * BLK
FTOT = 3 * BLK
FSPLIT = 2180  # first-piece DMA covers granules 0-1 of every tap


def _tap_off(dy, dx):
    if dx == -1:
        return B1 + 64 + 64 * dy
    if dx == 1:
        return B2 + 66 + 64 * dy
    return B3 + 64 + 64 * dy


# PE taps (x-shifted + center, fp16 diag matmuls); off-PE taps ride DVE
# affine_then_add ops that fuse tap multiply with the psum merge.
PTAPS = [(-1, -1), (-1, 1), (0, -1), (0, 0), (0, 1), (1, -1), (1, 1)]
OTAPS = [(-1, 0), (1, 0)]
NPT = len(PTAPS)
PTAP_OFFS = [_tap_off(*t) for t in PTAPS]
for _o in PTAP_OFFS:
    assert _o % 2 == 0, _o
OTAP_OFFS = [_tap_off(*t) for t in OTAPS]

PQW = 1 + HW + 1  # padded Pq row width


def _bilinear_matrix(n_in: int, n_out: int) -> np.ndarray:
    U = np.zeros((n_out, n_in), np.float64)
    s = n_in / n_out
    for i in range(n_out):
        c = (i + 0.5) * s - 0.5
        lo = int(np.floor(c))
        f = c - lo
        for idx, wt in ((lo, 1.0 - f), (lo + 1, f)):
            U[i, min(max(idx, 0), n_in - 1)] += wt
    return U


def _host_prep(f_e, all_prototypes, w_head, b_head):
    f_e = np.asarray(f_e, np.float32)
    ap = np.asarray(all_prototypes, np.float32)
    w_head = np.asarray(w_head, np.float32)
    b_val = float(np.asarray(b_head).reshape(-1)[0])

    # ---- flat fp16 variants ----
    f16 = f_e.astype(np.float16)  # [BS, C, H, W]
    z63 = f16.copy()
    z63[:, :, :, 63] = 0
    z0 = f16.copy()
    z0[:, :, :, 0] = 0
    fblk = np.zeros((BS, NCT, 128, FTOT), np.float16)
    for ct in range(NCT):
        sl = slice(ct * 128, (ct + 1) * 128)
        fblk[:, ct, :, B1 + 65: B1 + 65 + HW] = z63[:, sl].reshape(BS, 128, HW)
        fblk[:, ct, :, B2 + 65: B2 + 65 + HW] = z0[:, sl].reshape(BS, 128, HW)
        fblk[:, ct, :, B3 + 64: B3 + 64 + HW] = f16[:, sl].reshape(BS, 128, HW)

    # ---- conv weights: D0 = w0-w2, D1 = w1-w2 (fp16) ----
    # ap[s, o*9+t, b, c] -> wm[b, s, o, t(9), c]
    wm = ap.transpose(2, 0, 1, 3).reshape(BS, STEPS, NO, 9, C)
    v = np.stack([wm[:, :, 0] - wm[:, :, 2], wm[:, :, 1] - wm[:, :, 2]], axis=2)
    vf = v.astype(np.float16).astype(np.float32)  # [BS, S, 2, 9, C]

    tapidx = lambda dy, dx: (dy + 1) * 3 + (dx + 1)

    # PE-tap diag stationaries [BS, S, NCT, 2conv, 7tap, 128, 128] fp16
    diag = np.zeros((BS, STEPS, NCT, 2, NPT, 128, 128), np.float16)
    cidx = np.arange(128)
    for ct in range(NCT):
        sl = slice(ct * 128, (ct + 1) * 128)
        for pi, tp in enumerate(PTAPS):
            wa = vf[:, :, :, tapidx(*tp), sl]  # [BS, S, 2, 128]
            diag[:, :, ct, :, pi, cidx, cidx] = wa.transpose(3, 0, 1, 2).astype(np.float16)
    # reorder for per-(step,ct) DMA: -> [BS, S, NCT, 128, 2, 7, 128]
    diag = np.ascontiguousarray(diag.transpose(0, 1, 2, 5, 3, 4, 6))

    # fused-tap scalars for OTAPS: [BS, S, 128, NCT*2conv*3tap] f32
    NOT = len(OTAPS)
    wdy = np.zeros((BS, STEPS, 128, NCT * 2 * NOT), np.float32)
    for ct in range(NCT):
        sl = slice(ct * 128, (ct + 1) * 128)
        for cv in range(2):
            for i, (dy, dx) in enumerate(OTAPS):
                wdy[:, :, :, (ct * 2 + cv) * NOT + i] = vf[:, :, cv, tapidx(dy, dx), sl]

    # R2-head stationaries: u = w_head * w2; ug[b, s, ct, dy, c128, 65] fp16
    # (columns 0/32/64 hold the dx=-1/0/+1 weights so P rows land on
    #  partitions 0/32/64, which are legal K-tile bases for the shift matmuls)
    u = wm[:, :, 2] * w_head[None, None, None, :]  # [BS, S, 9tap, C]
    ug = np.zeros((BS, STEPS, NCT, 3, 128, 65), np.float16)
    for ct in range(NCT):
        sl = slice(ct * 128, (ct + 1) * 128)
        for dyi in range(3):
            for dxi in range(3):
                ug[:, :, ct, dyi, :, 32 * dxi] = u[:, :, dyi * 3 + dxi, sl].astype(np.float16)

    absw = np.abs(w_head).astype(np.float64)
    invw = np.where(absw > 0, 1.0 / np.maximum(absw, 1e-30), 1.0e30)
    invw = np.minimum(invw, 1.0e30).astype(np.float32)
    signw = np.sign(w_head).astype(np.float16)
    invw_t = np.ascontiguousarray(invw.reshape(NCT, 128, 1))
    signw_t = np.ascontiguousarray(signw.reshape(NCT, 128, 1))

    ut = _bilinear_matrix(H, HO).T.astype(np.float16)  # [64, 512]
    eye = np.eye(128, dtype=np.float16)
    ones3 = np.zeros((65, 1), np.float16)
    ones3[[0, 32, 64], 0] = 1.0

    in_maps = []
    for b in range(BS):
        in_maps.append(
            {
                "fblk": np.ascontiguousarray(fblk[b]),
                "diag": np.ascontiguousarray(diag[b]),
                "wdy": np.ascontiguousarray(wdy[b]),
                "ug": np.ascontiguousarray(ug[b]),
                "invw": invw_t,
                "signw": signw_t,
                "ut": ut,
                "eye": eye,
                "ones3": ones3,
            }
        )
    return in_maps, b_val


def _build_nc(b_val: float) -> bass.Bass:
    nc = bacc.Bacc(None, target_bir_lowering=False)
    fblk_d = nc.declare_dram_parameter("fblk", [NCT, 128, FTOT], F16, isOutput=False)
    diag_d = nc.declare_dram_parameter("diag", [STEPS, NCT, 128, 2 * NPT * 128], F16, isOutput=False)
    wdy_d = nc.declare_dram_parameter("wdy", [STEPS, 128, NCT * 2 * 2], F32, isOutput=False)
    ug_d = nc.declare_dram_parameter("ug", [STEPS, NCT, 3, 128, 65], F16, isOutput=False)
    invw_d = nc.declare_dram_parameter("invw", [NCT, 128, 1], F32, isOutput=False)
    signw_d = nc.declare_dram_parameter("signw", [NCT, 128, 1], F16, isOutput=False)
    ut_d = nc.declare_dram_parameter("ut", [64, WO], F16, isOutput=False)
    eye_d = nc.declare_dram_parameter("eye", [128, 128], F16, isOutput=False)
    ones3_d = nc.declare_dram_parameter("ones3", [65, 1], F16, isOutput=False)
    out_d = nc.declare_dram_parameter("out", [STEPS, HO, WO], F16, isOutput=True)

    with tile.TileContext(nc) as tc, ExitStack() as ctx:
        const = ctx.enter_context(tc.tile_pool(name="const", bufs=1))
        fpool = ctx.enter_context(tc.tile_pool(name="fpool", bufs=1))
        dpool = ctx.enter_context(tc.tile_pool(name="dpool", bufs=2))
        upool = ctx.enter_context(tc.tile_pool(name="upool", bufs=2))
        vpool = ctx.enter_context(tc.tile_pool(name="vpool", bufs=2))
        qpool = ctx.enter_context(tc.tile_pool(name="qpool", bufs=1))
        opool = ctx.enter_context(tc.tile_pool(name="opool", bufs=2))
        ps_d = ctx.enter_context(tc.tile_pool(name="ps_d", bufs=2, space="PSUM"))
        ps_p = ctx.enter_context(tc.tile_pool(name="ps_p", bufs=2, space="PSUM"))
        ps_head = ctx.enter_context(tc.tile_pool(name="ps_head", bufs=2, space="PSUM"))

        # ---- first-needed data first: step-0/ct-0 weights, then features ----
        dg00 = dpool.tile([128, 2 * NPT * 128], F16, tag="diag")
        nc.sync.dma_start(out=dg00[:], in_=diag_d[0, 0])
        fsb = []
        for ct in range(NCT):
            t = fpool.tile([128, FTOT], F16, tag=f"f{ct}")
            fsb.append(t)
        # staged feature DMA on the (otherwise idle) GpSimd DMA queue so the
        # step-0 weight DMAs on the sync queue never wait behind it
        def _fpiece(ct, lo, hi):
            nc.gpsimd.dma_start(
                out=fsb[ct][:].rearrange("p (b x) -> p b x", b=3)[:, :, lo:hi],
                in_=fblk_d[ct].rearrange("p (b x) -> p b x", b=3)[:, :, lo:hi],
            )

        _fpiece(0, 0, 1160)
        # prefetch the rest of step 0's weights before the bulk features
        dg01 = dpool.tile([128, 2 * NPT * 128], F16, tag="diag")
        nc.sync.dma_start(out=dg01[:], in_=diag_d[0, 1])
        ug00 = []
        for dyi in range(3):
            t = dpool.tile([128, 65], F16, tag=f"ug{dyi}")
            nc.sync.dma_start(out=t[:], in_=ug_d[0, 0, dyi])
            ug00.append(t)
        _fpiece(0, 1160, FSPLIT)
        _fpiece(1, 0, FSPLIT)
        _fpiece(0, FSPLIT, BLK)
        _fpiece(1, FSPLIT, BLK)

        # ---- constants ----
        ut_sb = const.tile([64, WO], F16, tag="ut")
        nc.sync.dma_start(out=ut_sb[:], in_=ut_d[:])
        eye_sb = const.tile([128, 128], F16, tag="eye")
        nc.sync.dma_start(out=eye_sb[:], in_=eye_d[:])
        ones3_sb = const.tile([65, 1], F16, tag="ones3")
        nc.sync.dma_start(out=ones3_sb[:], in_=ones3_d[:])
        invw_sb, signw_sb = [], []
        for ct in range(NCT):
            t = const.tile([128, 1], F32, tag=f"invw{ct}")
            nc.sync.dma_start(out=t[:], in_=invw_d[ct])
            invw_sb.append(t)
            t = const.tile([128, 1], F16, tag=f"signw{ct}")
            nc.sync.dma_start(out=t[:], in_=signw_d[ct])
            signw_sb.append(t)

        def emit_head(s, rsp_ct, pq):
            # ---- head: dmap chunks = R2h shifts + sign(w)-weighted t sums ----
            dmY = opool.tile([64, 64], F16, tag="dmY")
            for k in range(HW // 512):
                pd = ps_head.tile([1, 512], F32, tag="hps")
                for j in range(3):
                    base = 32 * j
                    nc.tensor.matmul(
                        pd[:],
                        eye_sb[0:65, base: base + 1],
                        pq[0:65, j + k * 512: j + k * 512 + 512],
                        start=(j == 0),
                        stop=False,
                    )
                for ct in range(NCT):
                    nc.tensor.matmul(
                        pd[:],
                        signw_sb[ct][:],
                        rsp_ct[ct][0][:, k * 512: (k + 1) * 512],
                        start=False,
                        stop=(ct == NCT - 1),
                    )
                dm1 = opool.tile([1, 512], F16, tag="dm1")
                nc.scalar.activation(dm1[:], pd[:], AF.Relu, bias=b_val)
                nc.sync.dma_start(
                    out=dmY[8 * k: 8 * k + 8, :],
                    in_=dm1[:].rearrange("p (y x) -> p y x", x=64),
                )
            # transpose -> x on partitions
            psT0 = ps_head.tile([64, 64], F16, tag="hps")
            nc.tensor.transpose(psT0[:], dmY[:], eye_sb[0:64, 0:64])
            dmX = opool.tile([64, 64], F16, tag="dmX")
            nc.vector.tensor_copy(dmX[:], psT0[:])

            # horizontal upsample in one matmul: H[y, X] (dmX stationary)
            ps_h = ps_head.tile([64, 512], F32, tag="hps")
            nc.tensor.matmul(ps_h[:], dmX[:], ut_sb[:], start=True, stop=True)
            h_sb = opool.tile([64, 512], F16, tag="h_sb")
            nc.scalar.activation(h_sb[:], ps_h[:], AF.Copy)
            # vertical upsample: out[Y, X] = sum_y ut[y, Y] * H[y, X]
            for yc in range(4):
                pv = ps_head.tile([128, 512], F32, tag="hps")
                nc.tensor.matmul(
                    pv[:],
                    ut_sb[:, yc * 128: (yc + 1) * 128],
                    h_sb[:],
                    start=True,
                    stop=True,
                )
                osb = opool.tile([128, 512], F16, tag="osb")
                nc.scalar.activation(osb[:], pv[:], AF.Copy)
                nc.sync.dma_start(out=out_d[s, yc * 128: (yc + 1) * 128, :], in_=osb[:])

        for s in range(STEPS):
            wy = dpool.tile([128, NCT * 2 * 2], F32, tag="wdy")
            nc.sync.dma_start(out=wy[:], in_=wdy_d[s])

            # R2-head stationaries for this step
            ug_sb = []
            for ct in range(NCT):
                if s == 0 and ct == 0:
                    ug_sb.append(ug00)
                else:
                    tiles = []
                    for dyi in range(3):
                        t = dpool.tile([128, 65], F16, tag=f"ug{ct}_{dyi}" if ct else f"ug{dyi}")
                        nc.sync.dma_start(out=t[:], in_=ug_d[s, ct, dyi])
                        tiles.append(t)
                    ug_sb.append(tiles)

            # ---- convs (PE taps + fused DVE taps/merge) + R2h P-pass ----
            rsp_ct = []
            pq = qpool.tile([65, PQW], F16, tag="pq")
            for ct in range(NCT):
                if s == 0 and ct == 0:
                    dg = dg00
                elif s == 0 and ct == 1:
                    dg = dg01
                else:
                    dg = dpool.tile([128, 2 * NPT * 128], F16, tag="diag")
                    nc.sync.dma_start(out=dg[:], in_=diag_d[s, ct])
                dgv = dg[:].rearrange("p (c pt x) -> p c pt x", c=2, pt=NPT)
                rsp = []
                for cv in range(2):
                    r = vpool.tile([128, HW + 32 * cv], F16, tag=f"rsp{ct}{cv}")
                    rsp.append(r)
                rsp_ct.append(rsp)
                for g in range(NGR):
                    gsl = slice(g * GR, (g + 1) * GR)
                    for cv in range(2):
                        acc = ps_d.tile([128, GR], F32, tag="acc")
                        for pi in range(NPT):
                            stat = dgv[:, cv, pi, :]
                            off = PTAP_OFFS[pi] + g * GR
                            for sub in range(GR // 512):
                                nc.tensor.matmul(
                                    acc[:, sub * 512: (sub + 1) * 512],
                                    stat,
                                    fsb[ct][:, off + sub * 512: off + (sub + 1) * 512],
                                    start=(pi == 0),
                                    stop=(pi == NPT - 1),
                                )
                        # fused off-PE taps + psum merge on DVE
                        wix = (ct * 2 + cv) * 2
                        r = rsp[cv]
                        nc.vector.affine_then_add(
                            out=r[:, gsl],
                            in0=fsb[ct][:, OTAP_OFFS[0] + g * GR: OTAP_OFFS[0] + (g + 1) * GR],
                            in1=acc[:],
                            scale=wy[:, wix: wix + 1],
                            bias=0.0,
                        )
                        nc.vector.affine_then_add(
                            out=r[:, gsl],
                            in0=fsb[ct][:, OTAP_OFFS[1] + g * GR: OTAP_OFFS[1] + (g + 1) * GR],
                            in1=r[:, gsl],
                            scale=wy[:, wix + 1: wix + 2],
                            bias=0.0,
                        )
            # P psum chunks: [65, 512] x8, 6 matmuls each (3 dy x 2 ct)
            for k in range(HW // 512):
                pp = ps_p.tile([65, 512], F32, tag="pp")
                first = True
                for ct in range(NCT):
                    for dyi, dy in enumerate((-1, 0, 1)):
                        off = B3 + 64 + 64 * dy + k * 512
                        nc.tensor.matmul(
                            pp[:],
                            ug_sb[ct][dyi][:],
                            fsb[ct][:, off: off + 512],
                            start=first,
                            stop=(ct == NCT - 1 and dyi == 2),
                        )
                        first = False
                nc.scalar.activation(pq[:, 1 + k * 512: 1 + (k + 1) * 512], pp[:], AF.Copy)
            # edge fixes on Pq: zero pads (all rows: avoid NaN*0 from garbage)
            nc.vector.memset(pq[0:65, 0:1], 0.0)
            nc.vector.memset(pq[0:65, PQW - 1: PQW], 0.0)
            pqv0 = pq[0:1, 1: 1 + HW].rearrange("p (y x) -> p y x", x=64)
            nc.vector.memset(pqv0[:, :, 63:64], 0.0)
            pqv2 = pq[64:65, 1: 1 + HW].rearrange("p (y x) -> p y x", x=64)
            nc.vector.memset(pqv2[:, :, 0:1], 0.0)

            # ---- softmax chain (in place, halves) ----
            e_ct = []
            hsls = [slice(0, HW // 2), slice(HW // 2, HW)]
            for ct in range(NCT):
                e0 = upool.tile([128, HW + 32], F16, tag=f"e0{ct}")
                e1 = upool.tile([128, HW + 64], F16, tag=f"e1{ct}")
                e_ct.append((e0, e1))
                for hs in hsls:
                    nc.scalar.activation(e0[:, hs], rsp_ct[ct][0][:, hs], AF.Exp)
                    nc.scalar.activation(e1[:, hs], rsp_ct[ct][1][:, hs], AF.Exp)
            for ct in range(NCT):
                D0r, D1r = rsp_ct[ct]
                e0, e1 = e_ct[ct]
                for hs in hsls:
                    nc.vector.tensor_mul(D0r[:, hs], D0r[:, hs], e0[:, hs])
                    nc.vector.tensor_mul(D1r[:, hs], D1r[:, hs], e1[:, hs])
                    nc.vector.tensor_add(D0r[:, hs], D0r[:, hs], D1r[:, hs])
                    nc.vector.tensor_add(e0[:, hs], e0[:, hs], e1[:, hs])
            for ct in range(NCT):
                e0, e1 = e_ct[ct]
                nc.scalar.activation(e1[:, 0:HW], e0[:, 0:HW], AF.Ln, scale=invw_sb[ct][:, 0:1], bias=invw_sb[ct][:, 0:1])
            for ct in range(NCT):
                e0, e1 = e_ct[ct]
                nc.scalar.activation(e0[:, 0:HW], e1[:, 0:HW], AF.Exp, scale=-1.0)
            for ct in range(NCT):
                D0r, D1r = rsp_ct[ct]
                e0, e1 = e_ct[ct]
                for hs in hsls:
                    nc.vector.tensor_mul(D0r[:, hs], D0r[:, hs], e0[:, hs])

            emit_head(s, rsp_ct, pq)

    nc.compile()
    return nc


_CACHE = {}


def _get_nc(b_val: float) -> bass.Bass:
    key = round(b_val, 12)
    if key not in _CACHE:
        _CACHE[key] = _build_nc(b_val)
    return _CACHE[key]


def kernel(f_e, all_prototypes, w_head, b_head):
    in_maps, b_val = _host_prep(f_e, all_prototypes, w_head, b_head)
    nc = _get_nc(b_val)
    res = run_bass_kernel_spmd(nc, in_maps, list(range(NCORES)), trace=False)
    outs = [res.results[b]["out"].reshape(STEPS, 1, HO, WO) for b in range(BS)]
    full = np.stack(outs, axis=1)  # [STEPS, BS, 1, HO, WO]
    return full.astype(np.float32)


# revision 35
# speedup vs baseline: 1.2193x; 1.1333x over previous
"""LOCA-style kernel for Trainium2, data-parallel over batch on 8 NeuronCores.

Per core (one batch element), per step:
  - depthwise 3x3 correlation for D0=conv(w0-w2), D1=conv(w1-w2): 7 taps
    as fp16 diagonal-weight PE matmuls on a flat-raster feature map
    (zero-edge-column variants give exact padding); the 2 remaining (dy,0)
    taps are fused with the PSUM merge via DVE affine_then_add
    (out = f_shift*wdy + acc), one op per tap per granule.
  - R2's head contribution is linear, so it never materializes per-channel:
    P[dx, n] = sum_dy sum_c (w_head*w2)[c,dy,dx] * f[c, n+64*dy] via 6
    matmul passes with [128 -> {0,32,64}] stationaries, then 3 tiny K=1
    shift-matmuls per 512-chunk accumulate P's dx-shifts straight into the
    head PSUM (edge wrap killed by strided memsets on the SBUF copy).
  - softmax-weighted object sum via the shift identity
      red = R2 + (D0*e0 + D1*e1) / (1 + e0 + e1),  e_i = exp(D_i)
    with exps/Ln on ScalarE (|w_head| folded into the Ln scale/bias),
    products and sums on DVE fp16 2x mode, all in-place on 4 full-map tags.
  - 1x1 head with sign(w_head) stationary + ReLU + 8x bilinear upsample
    as two separable matmul passes. Output fp16, upcast on host.
"""

import sys

sys.path.insert(0, "/opt/trn_rl_repo")

import numpy as np
from contextlib import ExitStack

import concourse.bass as bass
import concourse.mybir as mybir
from concourse import bacc, tile
from concourse.bass_utils import run_bass_kernel_spmd

BS, C, H, W = 8, 256, 64, 64
STEPS, NO = 3, 3
RED = 8
HO, WO = H * RED, W * RED  # 512, 512
NCORES = 8
NCT = 2
HW = H * W  # 4096
GR = 1024  # conv psum granule (pixels)
NGR = HW // GR  # 4
F16 = mybir.dt.float16
F32 = mybir.dt.float32
AF = mybir.ActivationFunctionType
ALU = mybir.AluOpType

# flat f8 layout: one tile [128, 3*BLK] per ct;
#   block 0 (Vz63): col x=63 zeroed, x-origin at 65 (for dx=-1 taps)
#   block 1 (Vz0):  col x=0 zeroed, x-origin at 65 (for dx=+1 taps)
#   block 2 (V0):   full map, x-origin at 64 (for dx=0 taps)
BLK = 4232  # 64 head pad + 4096 + 72 tail pad, even
B1, B2, B3 = 0, BLK, 2 * BLK
FTOT = 3 * BLK
FSPLIT = 2180  # first-piece DMA covers granules 0-1 of every tap


def _tap_off(dy, dx):
    if dx == -1:
        return B1 + 64 + 64 * dy
    if dx == 1:
        return B2 + 66 + 64 * dy
    return B3 + 64 + 64 * dy


# PE taps (x-shifted + center, fp16 diag matmuls); off-PE taps ride DVE
# affine_then_add ops that fuse tap multiply with the psum merge.
PTAPS = [(-1, -1), (-1, 1), (0, -1), (0, 0), (0, 1), (1, -1), (1, 1)]
OTAPS = [(-1, 0), (1, 0)]
NPT = len(PTAPS)
PTAP_OFFS = [_tap_off(*t) for t in PTAPS]
for _o in PTAP_OFFS:
    assert _o % 2 == 0, _o
OTAP_OFFS = [_tap_off(*t) for t in OTAPS]

PQW = 1 + HW + 1  # padded Pq row width


def _bilinear_matrix(n_in: int, n_out: int) -> np.ndarray:
    U = np.zeros((n_out, n_in), np.float64)
    s = n_in / n_out
    for i in range(n_out):
        c = (i + 0.5) * s - 0.5
        lo = int(np.floor(c))
        f = c - lo
        for idx, wt in ((lo, 1.0 - f), (lo + 1, f)):
            U[i, min(max(idx, 0), n_in - 1)] += wt
    return U


def _host_prep(f_e, all_prototypes, w_head, b_head):
    f_e = np.asarray(f_e, np.float32)
    ap = np.asarray(all_prototypes, np.float32)
    w_head = np.asarray(w_head, np.float32)
    b_val = float(np.asarray(b_head).reshape(-1)[0])

    # ---- flat fp16 variants ----
    f16 = f_e.astype(np.float16)  # [BS, C, H, W]
    z63 = f16.copy()
    z63[:, :, :, 63] = 0
    z0 = f16.copy()
    z0[:, :, :, 0] = 0
    fblk = np.zeros((BS, NCT, 128, FTOT), np.float16)
    for ct in range(NCT):
        sl = slice(ct * 128, (ct + 1) * 128)
        fblk[:, ct, :, B1 + 65: B1 + 65 + HW] = z63[:, sl].reshape(BS, 128, HW)
        fblk[:, ct, :, B2 + 65: B2 + 65 + HW] = z0[:, sl].reshape(BS, 128, HW)
        fblk[:, ct, :, B3 + 64: B3 + 64 + HW] = f16[:, sl].reshape(BS, 128, HW)

    # ---- conv weights: D0 = w0-w2, D1 = w1-w2 (fp16) ----
    # ap[s, o*9+t, b, c] -> wm[b, s, o, t(9), c]
    wm = ap.transpose(2, 0, 1, 3).reshape(BS, STEPS, NO, 9, C)
    v = np.stack([wm[:, :, 0] - wm[:, :, 2], wm[:, :, 1] - wm[:, :, 2]], axis=2)
    vf = v.astype(np.float16).astype(np.float32)  # [BS, S, 2, 9, C]

    tapidx = lambda dy, dx: (dy + 1) * 3 + (dx + 1)

    # PE-tap diag stationaries [BS, S, NCT, 2conv, 7tap, 128, 128] fp16
    diag = np.zeros((BS, STEPS, NCT, 2, NPT, 128, 128), np.float16)
    cidx = np.arange(128)
    for ct in range(NCT):
        sl = slice(ct * 128, (ct + 1) * 128)
        for pi, tp in enumerate(PTAPS):
            wa = vf[:, :, :, tapidx(*tp), sl]  # [BS, S, 2, 128]
            diag[:, :, ct, :, pi, cidx, cidx] = wa.transpose(3, 0, 1, 2).astype(np.float16)
    # reorder for per-(step,ct) DMA: -> [BS, S, NCT, 128, 2, 7, 128]
    diag = np.ascontiguousarray(diag.transpose(0, 1, 2, 5, 3, 4, 6))

    # fused-tap scalars for OTAPS: [BS, S, 128, NCT*2conv*3tap] f32
    NOT = len(OTAPS)
    wdy = np.zeros((BS, STEPS, 128, NCT * 2 * NOT), np.float32)
    for ct in range(NCT):
        sl = slice(ct * 128, (ct + 1) * 128)
        for cv in range(2):
            for i, (dy, dx) in enumerate(OTAPS):
                wdy[:, :, :, (ct * 2 + cv) * NOT + i] = vf[:, :, cv, tapidx(dy, dx), sl]

    # R2-head stationaries: u = w_head * w2; ug[b, s, ct, dy, c128, 65] fp16
    # (columns 0/32/64 hold the dx=-1/0/+1 weights so P rows land on
    #  partitions 0/32/64, which are legal K-tile bases for the shift matmuls)
    u = wm[:, :, 2] * w_head[None, None, None, :]  # [BS, S, 9tap, C]
    ug = np.zeros((BS, STEPS, NCT, 3, 128, 65), np.float16)
    for ct in range(NCT):
        sl = slice(ct * 128, (ct + 1) * 128)
        for dyi in range(3):
            for dxi in range(3):
                ug[:, :, ct, dyi, :, 32 * dxi] = u[:, :, dyi * 3 + dxi, sl].astype(np.float16)

    absw = np.abs(w_head).astype(np.float64)
    invw = np.where(absw > 0, 1.0 / np.maximum(absw, 1e-30), 1.0e30)
    invw = np.minimum(invw, 1.0e30).astype(np.float32)
    signw = np.sign(w_head).astype(np.float16)
    invw_t = np.ascontiguousarray(invw.reshape(NCT, 128, 1))
    signw_t = np.ascontiguousarray(signw.reshape(NCT, 128, 1))

    ut = _bilinear_matrix(H, HO).T.astype(np.float16)  # [64, 512]
    eye = np.eye(128, dtype=np.float16)
    ones3 = np.zeros((65, 1), np.float16)
    ones3[[0, 32, 64], 0] = 1.0

    in_maps = []
    for b in range(BS):
        in_maps.append(
            {
                "fblk": np.ascontiguousarray(fblk[b]),
                "diag": np.ascontiguousarray(diag[b]),
                "wdy": np.ascontiguousarray(wdy[b]),
                "ug": np.ascontiguousarray(ug[b]),
                "invw": invw_t,
                "signw": signw_t,
                "ut": ut,
                "eye": eye,
                "ones3": ones3,
            }
        )
    return in_maps, b_val


def _build_nc(b_val: float) -> bass.Bass:
    nc = bacc.Bacc(None, target_bir_lowering=False)
    fblk_d = nc.declare_dram_parameter("fblk", [NCT, 128, FTOT], F16, isOutput=False)
    diag_d = nc.declare_dram_parameter("diag", [STEPS, NCT, 128, 2 * NPT * 128], F16, isOutput=False)
    wdy_d = nc.declare_dram_parameter("wdy", [STEPS, 128, NCT * 2 * 2], F32, isOutput=False)
    ug_d = nc.declare_dram_parameter("ug", [STEPS, NCT, 3, 128, 65], F16, isOutput=False)
    invw_d = nc.declare_dram_parameter("invw", [NCT, 128, 1], F32, isOutput=False)
    signw_d = nc.declare_dram_parameter("signw", [NCT, 128, 1], F16, isOutput=False)
    ut_d = nc.declare_dram_parameter("ut", [64, WO], F16, isOutput=False)
    eye_d = nc.declare_dram_parameter("eye", [128, 128], F16, isOutput=False)
    ones3_d = nc.declare_dram_parameter("ones3", [65, 1], F16, isOutput=False)
    out_d = nc.declare_dram_parameter("out", [STEPS, HO, WO], F16, isOutput=True)

    with tile.TileContext(nc) as tc, ExitStack() as ctx:
        const = ctx.enter_context(tc.tile_pool(name="const", bufs=1))
        fpool = ctx.enter_context(tc.tile_pool(name="fpool", bufs=1))
        dpool = ctx.enter_context(tc.tile_pool(name="dpool", bufs=2))
        upool = ctx.enter_context(tc.tile_pool(name="upool", bufs=2))
        vpool = ctx.enter_context(tc.tile_pool(name="vpool", bufs=2))
        qpool = ctx.enter_context(tc.tile_pool(name="qpool", bufs=1))
        opool = ctx.enter_context(tc.tile_pool(name="opool", bufs=2))
        ps_d = ctx.enter_context(tc.tile_pool(name="ps_d", bufs=2, space="PSUM"))
        ps_p = ctx.enter_context(tc.tile_pool(name="ps_p", bufs=1, space="PSUM"))
        ps_head = ctx.enter_context(tc.tile_pool(name="ps_head", bufs=3, space="PSUM"))

        # ---- first-needed data first: step-0/ct-0 weights, then features ----
        dg00 = dpool.tile([128, 2 * NPT * 128], F16, tag="diag")
        nc.sync.dma_start(out=dg00[:], in_=diag_d[0, 0])
        fsb = []
        for ct in range(NCT):
            t = fpool.tile([128, FTOT], F16, tag=f"f{ct}")
            fsb.append(t)
        # staged feature DMA on the (otherwise idle) GpSimd DMA queue so the
        # step-0 weight DMAs on the sync queue never wait behind it
        def _fpiece(ct, lo, hi):
            nc.gpsimd.dma_start(
                out=fsb[ct][:].rearrange("p (b x) -> p b x", b=3)[:, :, lo:hi],
                in_=fblk_d[ct].rearrange("p (b x) -> p b x", b=3)[:, :, lo:hi],
            )

        _fpiece(0, 0, 1160)
        # prefetch the rest of step 0's weights before the bulk features
        dg01 = dpool.tile([128, 2 * NPT * 128], F16, tag="diag")
        nc.sync.dma_start(out=dg01[:], in_=diag_d[0, 1])
        ug00 = []
        for dyi in range(3):
            t = dpool.tile([128, 65], F16, tag=f"ug{dyi}")
            nc.sync.dma_start(out=t[:], in_=ug_d[0, 0, dyi])
            ug00.append(t)
        _fpiece(0, 1160, FSPLIT)
        _fpiece(1, 0, FSPLIT)
        _fpiece(0, FSPLIT, BLK)
        _fpiece(1, FSPLIT, BLK)

        # ---- constants ----
        ut_sb = const.tile([64, WO], F16, tag="ut")
        nc.sync.dma_start(out=ut_sb[:], in_=ut_d[:])
        eye_sb = const.tile([128, 128], F16, tag="eye")
        nc.sync.dma_start(out=eye_sb[:], in_=eye_d[:])
        ones3_sb = const.tile([65, 1], F16, tag="ones3")
        nc.sync.dma_start(out=ones3_sb[:], in_=ones3_d[:])
        invw_sb, signw_sb = [], []
        for ct in range(NCT):
            t = const.tile([128, 1], F32, tag=f"invw{ct}")
            nc.sync.dma_start(out=t[:], in_=invw_d[ct])
            invw_sb.append(t)
            t = const.tile([128, 1], F16, tag=f"signw{ct}")
            nc.sync.dma_start(out=t[:], in_=signw_d[ct])
            signw_sb.append(t)

        def emit_head(s, rsp_ct, pq):
            # ---- head: dmap chunks = R2h shifts + sign(w)-weighted t sums ----
            dmY = opool.tile([64, 64], F16, tag="dmY")
            for k in range(HW // 512):
                pd = ps_head.tile([1, 512], F32, tag="hps")
                for j in range(3):
                    base = 32 * j
                    nc.tensor.matmul(
                        pd[:],
                        eye_sb[0:65, base: base + 1],
                        pq[0:65, j + k * 512: j + k * 512 + 512],
                        start=(j == 0),
                        stop=False,
                    )
                for ct in range(NCT):
                    nc.tensor.matmul(
                        pd[:],
                        signw_sb[ct][:],
                        rsp_ct[ct][0][:, k * 512: (k + 1) * 512],
                        start=False,
                        stop=(ct == NCT - 1),
                    )
                dm1 = opool.tile([1, 512], F16, tag="dm1")
                nc.scalar.activation(dm1[:], pd[:], AF.Relu, bias=b_val)
                nc.sync.dma_start(
                    out=dmY[8 * k: 8 * k + 8, :],
                    in_=dm1[:].rearrange("p (y x) -> p y x", x=64),
                )
            # transpose -> x on partitions
            psT0 = ps_head.tile([64, 64], F16, tag="hps")
            nc.tensor.transpose(psT0[:], dmY[:], eye_sb[0:64, 0:64])
            dmX = opool.tile([64, 64], F16, tag="dmX")
            nc.vector.tensor_copy(dmX[:], psT0[:])

            # horizontal upsample in one matmul: H[y, X] (dmX stationary)
            ps_h = ps_head.tile([64, 512], F32, tag="hps")
            nc.tensor.matmul(ps_h[:], dmX[:], ut_sb[:], start=True, stop=True)
            h_sb = opool.tile([64, 512], F16, tag="h_sb")
            nc.scalar.activation(h_sb[:], ps_h[:], AF.Copy)
            # vertical upsample: out[Y, X] = sum_y ut[y, Y] * H[y, X]
            for yc in range(4):
                pv = ps_head.tile([128, 512], F32, tag="hps")
                nc.tensor.matmul(
                    pv[:],
                    ut_sb[:, yc * 128: (yc + 1) * 128],
                    h_sb[:],
                    start=True,
                    stop=True,
                )
                osb = opool.tile([128, 512], F16, tag="osb")
                nc.scalar.activation(osb[:], pv[:], AF.Copy)
                nc.sync.dma_start(out=out_d[s, yc * 128: (yc + 1) * 128, :], in_=osb[:])

        for s in range(STEPS):
            wy = dpool.tile([128, NCT * 2 * 2], F32, tag="wdy")
            nc.sync.dma_start(out=wy[:], in_=wdy_d[s])

            # R2-head stationaries for this step
            ug_sb = []
            for ct in range(NCT):
                if s == 0 and ct == 0:
                    ug_sb.append(ug00)
                else:
                    tiles = []
                    for dyi in range(3):
                        t = dpool.tile([128, 65], F16, tag=f"ug{ct}_{dyi}" if ct else f"ug{dyi}")
                        nc.sync.dma_start(out=t[:], in_=ug_d[s, ct, dyi])
                        tiles.append(t)
                    ug_sb.append(tiles)

            # ---- convs (PE taps + fused DVE taps/merge) + R2h P-pass ----
            rsp_ct = []
            pq = qpool.tile([65, PQW], F16, tag="pq")
            for ct in range(NCT):
                if s == 0 and ct == 0:
                    dg = dg00
                elif s == 0 and ct == 1:
                    dg = dg01
                else:
                    dg = dpool.tile([128, 2 * NPT * 128], F16, tag="diag")
                    nc.sync.dma_start(out=dg[:], in_=diag_d[s, ct])
                dgv = dg[:].rearrange("p (c pt x) -> p c pt x", c=2, pt=NPT)
                rsp = []
                for cv in range(2):
                    r = vpool.tile([128, HW + 32 * cv], F16, tag=f"rsp{ct}{cv}")
                    rsp.append(r)
                rsp_ct.append(rsp)
                for g in range(NGR):
                    gsl = slice(g * GR, (g + 1) * GR)
                    for cv in range(2):
                        acc = ps_d.tile([128, GR], F32, tag="acc")
                        for pi in range(NPT):
                            stat = dgv[:, cv, pi, :]
                            off = PTAP_OFFS[pi] + g * GR
                            for sub in range(GR // 512):
                                nc.tensor.matmul(
                                    acc[:, sub * 512: (sub + 1) * 512],
                                    stat,
                                    fsb[ct][:, off + sub * 512: off + (sub + 1) * 512],
                                    start=(pi == 0),
                                    stop=(pi == NPT - 1),
                                )
                        # fused off-PE taps + psum merge on DVE
                        wix = (ct * 2 + cv) * 2
                        r = rsp[cv]
                        nc.vector.affine_then_add(
                            out=r[:, gsl],
                            in0=fsb[ct][:, OTAP_OFFS[0] + g * GR: OTAP_OFFS[0] + (g + 1) * GR],
                            in1=acc[:],
                            scale=wy[:, wix: wix + 1],
                            bias=0.0,
                        )
                        nc.vector.affine_then_add(
                            out=r[:, gsl],
                            in0=fsb[ct][:, OTAP_OFFS[1] + g * GR: OTAP_OFFS[1] + (g + 1) * GR],
                            in1=r[:, gsl],
                            scale=wy[:, wix + 1: wix + 2],
                            bias=0.0,
                        )
            # P psum chunks: [65, 512] x8, 6 matmuls each (3 dy x 2 ct)
            for k in range(HW // 512):
                pp = ps_p.tile([65, 512], F32, tag="pp")
                first = True
                for ct in range(NCT):
                    for dyi, dy in enumerate((-1, 0, 1)):
                        off = B3 + 64 + 64 * dy + k * 512
                        nc.tensor.matmul(
                            pp[:],
                            ug_sb[ct][dyi][:],
                            fsb[ct][:, off: off + 512],
                            start=first,
                            stop=(ct == NCT - 1 and dyi == 2),
                        )
                        first = False
                nc.scalar.activation(pq[:, 1 + k * 512: 1 + (k + 1) * 512], pp[:], AF.Copy)
            # edge fixes on Pq: zero pads (all rows: avoid NaN*0 from garbage)
            nc.vector.memset(pq[0:65, 0:1], 0.0)
            nc.vector.memset(pq[0:65, PQW - 1: PQW], 0.0)
            pqv0 = pq[0:1, 1: 1 + HW].rearrange("p (y x) -> p y x", x=64)
            nc.vector.memset(pqv0[:, :, 63:64], 0.0)
            pqv2 = pq[64:65, 1: 1 + HW].rearrange("p (y x) -> p y x", x=64)
            nc.vector.memset(pqv2[:, :, 0:1], 0.0)

            # ---- softmax chain (in place, halves) ----
            e_ct = []
            hsls = [slice(0, HW // 2), slice(HW // 2, HW)]
            for ct in range(NCT):
                e0 = upool.tile([128, HW + 32], F16, tag=f"e0{ct}")
                e1 = upool.tile([128, HW + 64], F16, tag=f"e1{ct}")
                e_ct.append((e0, e1))
                for hs in hsls:
                    nc.scalar.activation(e0[:, hs], rsp_ct[ct][0][:, hs], AF.Exp)
                    nc.scalar.activation(e1[:, hs], rsp_ct[ct][1][:, hs], AF.Exp)
            for ct in range(NCT):
                D0r, D1r = rsp_ct[ct]
                e0, e1 = e_ct[ct]
                for hs in hsls:
                    nc.vector.tensor_mul(D0r[:, hs], D0r[:, hs], e0[:, hs])
                    nc.vector.tensor_mul(D1r[:, hs], D1r[:, hs], e1[:, hs])
                    nc.vector.tensor_add(D0r[:, hs], D0r[:, hs], D1r[:, hs])
                    nc.vector.tensor_add(e0[:, hs], e0[:, hs], e1[:, hs])
            lnsl = hsls if s == STEPS - 1 else [slice(0, HW)]
            for ct in range(NCT):
                e0, e1 = e_ct[ct]
                for hs in lnsl:
                    nc.scalar.activation(e1[:, hs], e0[:, hs], AF.Ln, scale=invw_sb[ct][:, 0:1], bias=invw_sb[ct][:, 0:1])
            for ct in range(NCT):
                e0, e1 = e_ct[ct]
                for hs in lnsl:
                    nc.scalar.activation(e0[:, hs], e1[:, hs], AF.Exp, scale=-1.0)
            for ct in range(NCT):
                D0r, D1r = rsp_ct[ct]
                e0, e1 = e_ct[ct]
                for hs in hsls:
                    nc.vector.tensor_mul(D0r[:, hs], D0r[:, hs], e0[:, hs])

            emit_head(s, rsp_ct, pq)

    nc.compile()
    return nc


_CACHE = {}


def _get_nc(b_val: float) -> bass.Bass:
    key = round(b_val, 12)
    if key not in _CACHE:
        _CACHE[key] = _build_nc(b_val)
    return _CACHE[key]


def kernel(f_e, all_prototypes, w_head, b_head):
    in_maps, b_val = _host_prep(f_e, all_prototypes, w_head, b_head)
    nc = _get_nc(b_val)
    res = run_bass_kernel_spmd(nc, in_maps, list(range(NCORES)), trace=False)
    outs = [res.results[b]["out"].reshape(STEPS, 1, HO, WO) for b in range(BS)]
    full = np.stack(outs, axis=1)  # [STEPS, BS, 1, HO, WO]
    return full.astype(np.float32)


# revision 36
# speedup vs baseline: 1.2567x; 1.0307x over previous
"""LOCA-style kernel for Trainium2, data-parallel over batch on 8 NeuronCores.

Per core (one batch element), per step:
  - depthwise 3x3 correlation for D0=conv(w0-w2), D1=conv(w1-w2): 7 taps
    as fp16 diagonal-weight PE matmuls on a flat-raster feature map
    (zero-edge-column variants give exact padding); the 2 remaining (dy,0)
    taps are fused with the PSUM merge via DVE affine_then_add
    (out = f_shift*wdy + acc), one op per tap per granule.
  - R2's head contribution is linear, so it never materializes per-channel:
    P[dx, n] = sum_dy sum_c (w_head*w2)[c,dy,dx] * f[c, n+64*dy] via 6
    matmul passes with [128 -> {0,32,64}] stationaries, then 3 tiny K=1
    shift-matmuls per 512-chunk accumulate P's dx-shifts straight into the
    head PSUM (edge wrap killed by strided memsets on the SBUF copy).
  - softmax-weighted object sum via the shift identity
      red = R2 + (D0*e0 + D1*e1) / (1 + e0 + e1),  e_i = exp(D_i)
    with exps/Ln on ScalarE (|w_head| folded into the Ln scale/bias),
    products and sums on DVE fp16 2x mode, all in-place on 4 full-map tags.
  - 1x1 head with sign(w_head) stationary + ReLU + 8x bilinear upsample
    as two separable matmul passes. Output fp16, upcast on host.
"""

import sys

sys.path.insert(0, "/opt/trn_rl_repo")

import numpy as np
from contextlib import ExitStack

import concourse.bass as bass
import concourse.mybir as mybir
from concourse import bacc, tile
from concourse.bass_utils import run_bass_kernel_spmd

BS, C, H, W = 8, 256, 64, 64
STEPS, NO = 3, 3
RED = 8
HO, WO = H * RED, W * RED  # 512, 512
NCORES = 8
NCT = 2
HW = H * W  # 4096
GR = 1024  # conv psum granule (pixels)
NGR = HW // GR  # 4
F16 = mybir.dt.float16
F32 = mybir.dt.float32
AF = mybir.ActivationFunctionType
ALU = mybir.AluOpType

# flat f8 layout: one tile [128, 3*BLK] per ct;
#   block 0 (Vz63): col x=63 zeroed, x-origin at 65 (for dx=-1 taps)
#   block 1 (Vz0):  col x=0 zeroed, x-origin at 65 (for dx=+1 taps)
#   block 2 (V0):   full map, x-origin at 64 (for dx=0 taps)
BLK = 4232  # 64 head pad + 4096 + 72 tail pad, even
B1, B2, B3 = 0, BLK, 2 * BLK
FTOT = 3 * BLK
FSPLIT = 2180  # first-piece DMA covers granules 0-1 of every tap


def _tap_off(dy, dx):
    if dx == -1:
        return B1 + 64 + 64 * dy
    if dx == 1:
        return B2 + 66 + 64 * dy
    return B3 + 64 + 64 * dy


# PE taps (x-shifted + center, fp16 diag matmuls); off-PE taps ride DVE
# affine_then_add ops that fuse tap multiply with the psum merge.
PTAPS = [(-1, -1), (-1, 1), (0, -1), (0, 0), (0, 1), (1, -1), (1, 1)]
OTAPS = [(-1, 0), (1, 0)]
NPT = len(PTAPS)
PTAP_OFFS = [_tap_off(*t) for t in PTAPS]
for _o in PTAP_OFFS:
    assert _o % 2 == 0, _o
OTAP_OFFS = [_tap_off(*t) for t in OTAPS]

PQW = 1 + HW + 1  # padded Pq row width


def _bilinear_matrix(n_in: int, n_out: int) -> np.ndarray:
    U = np.zeros((n_out, n_in), np.float64)
    s = n_in / n_out
    for i in range(n_out):
        c = (i + 0.5) * s - 0.5
        lo = int(np.floor(c))
        f = c - lo
        for idx, wt in ((lo, 1.0 - f), (lo + 1, f)):
            U[i, min(max(idx, 0), n_in - 1)] += wt
    return U


def _host_prep(f_e, all_prototypes, w_head, b_head):
    f_e = np.asarray(f_e, np.float32)
    ap = np.asarray(all_prototypes, np.float32)
    w_head = np.asarray(w_head, np.float32)
    b_val = float(np.asarray(b_head).reshape(-1)[0])

    # ---- flat fp16 variants ----
    f16 = f_e.astype(np.float16)  # [BS, C, H, W]
    z63 = f16.copy()
    z63[:, :, :, 63] = 0
    z0 = f16.copy()
    z0[:, :, :, 0] = 0
    fblk = np.zeros((BS, NCT, 128, FTOT), np.float16)
    for ct in range(NCT):
        sl = slice(ct * 128, (ct + 1) * 128)
        fblk[:, ct, :, B1 + 65: B1 + 65 + HW] = z63[:, sl].reshape(BS, 128, HW)
        fblk[:, ct, :, B2 + 65: B2 + 65 + HW] = z0[:, sl].reshape(BS, 128, HW)
        fblk[:, ct, :, B3 + 64: B3 + 64 + HW] = f16[:, sl].reshape(BS, 128, HW)

    # ---- conv weights: D0 = w0-w2, D1 = w1-w2 (fp16) ----
    # ap[s, o*9+t, b, c] -> wm[b, s, o, t(9), c]
    wm = ap.transpose(2, 0, 1, 3).reshape(BS, STEPS, NO, 9, C)
    v = np.stack([wm[:, :, 0] - wm[:, :, 2], wm[:, :, 1] - wm[:, :, 2]], axis=2)
    vf = v.astype(np.float16).astype(np.float32)  # [BS, S, 2, 9, C]

    tapidx = lambda dy, dx: (dy + 1) * 3 + (dx + 1)

    # PE-tap diag stationaries [BS, S, NCT, 2conv, 7tap, 128, 128] fp16
    diag = np.zeros((BS, STEPS, NCT, 2, NPT, 128, 128), np.float16)
    cidx = np.arange(128)
    for ct in range(NCT):
        sl = slice(ct * 128, (ct + 1) * 128)
        for pi, tp in enumerate(PTAPS):
            wa = vf[:, :, :, tapidx(*tp), sl]  # [BS, S, 2, 128]
            diag[:, :, ct, :, pi, cidx, cidx] = wa.transpose(3, 0, 1, 2).astype(np.float16)
    # reorder for per-(step,ct) DMA: -> [BS, S, NCT, 128, 2, 7, 128]
    diag = np.ascontiguousarray(diag.transpose(0, 1, 2, 5, 3, 4, 6))

    # fused-tap scalars for OTAPS: [BS, S, 128, NCT*2conv*3tap] f32
    NOT = len(OTAPS)
    wdy = np.zeros((BS, STEPS, 128, NCT * 2 * NOT), np.float32)
    for ct in range(NCT):
        sl = slice(ct * 128, (ct + 1) * 128)
        for cv in range(2):
            for i, (dy, dx) in enumerate(OTAPS):
                wdy[:, :, :, (ct * 2 + cv) * NOT + i] = vf[:, :, cv, tapidx(dy, dx), sl]

    # R2-head stationaries: u = w_head * w2; ug[b, s, ct, dy, c128, 65] fp16
    # (columns 0/32/64 hold the dx=-1/0/+1 weights so P rows land on
    #  partitions 0/32/64, which are legal K-tile bases for the shift matmuls)
    u = wm[:, :, 2] * w_head[None, None, None, :]  # [BS, S, 9tap, C]
    ug = np.zeros((BS, STEPS, NCT, 3, 128, 65), np.float16)
    for ct in range(NCT):
        sl = slice(ct * 128, (ct + 1) * 128)
        for dyi in range(3):
            for dxi in range(3):
                ug[:, :, ct, dyi, :, 32 * dxi] = u[:, :, dyi * 3 + dxi, sl].astype(np.float16)

    absw = np.abs(w_head).astype(np.float64)
    invw = np.where(absw > 0, 1.0 / np.maximum(absw, 1e-30), 1.0e30)
    invw = np.minimum(invw, 1.0e30).astype(np.float32)
    signw = np.sign(w_head).astype(np.float16)
    invw_t = np.ascontiguousarray(invw.reshape(NCT, 128, 1))
    signw_t = np.ascontiguousarray(signw.reshape(NCT, 128, 1))

    ut = _bilinear_matrix(H, HO).T.astype(np.float16)  # [64, 512]
    eye = np.eye(128, dtype=np.float16)
    ones3 = np.zeros((65, 1), np.float16)
    ones3[[0, 32, 64], 0] = 1.0

    in_maps = []
    for b in range(BS):
        in_maps.append(
            {
                "fblk": np.ascontiguousarray(fblk[b]),
                "diag": np.ascontiguousarray(diag[b]),
                "wdy": np.ascontiguousarray(wdy[b]),
                "ug": np.ascontiguousarray(ug[b]),
                "invw": invw_t,
                "signw": signw_t,
                "ut": ut,
                "eye": eye,
                "ones3": ones3,
            }
        )
    return in_maps, b_val


def _build_nc(b_val: float) -> bass.Bass:
    nc = bacc.Bacc(None, target_bir_lowering=False)
    fblk_d = nc.declare_dram_parameter("fblk", [NCT, 128, FTOT], F16, isOutput=False)
    diag_d = nc.declare_dram_parameter("diag", [STEPS, NCT, 128, 2 * NPT * 128], F16, isOutput=False)
    wdy_d = nc.declare_dram_parameter("wdy", [STEPS, 128, NCT * 2 * 2], F32, isOutput=False)
    ug_d = nc.declare_dram_parameter("ug", [STEPS, NCT, 3, 128, 65], F16, isOutput=False)
    invw_d = nc.declare_dram_parameter("invw", [NCT, 128, 1], F32, isOutput=False)
    signw_d = nc.declare_dram_parameter("signw", [NCT, 128, 1], F16, isOutput=False)
    ut_d = nc.declare_dram_parameter("ut", [64, WO], F16, isOutput=False)
    eye_d = nc.declare_dram_parameter("eye", [128, 128], F16, isOutput=False)
    ones3_d = nc.declare_dram_parameter("ones3", [65, 1], F16, isOutput=False)
    out_d = nc.declare_dram_parameter("out", [STEPS, HO, WO], F16, isOutput=True)

    with tile.TileContext(nc) as tc, ExitStack() as ctx:
        const = ctx.enter_context(tc.tile_pool(name="const", bufs=1))
        fpool = ctx.enter_context(tc.tile_pool(name="fpool", bufs=1))
        dpool = ctx.enter_context(tc.tile_pool(name="dpool", bufs=2))
        upool = ctx.enter_context(tc.tile_pool(name="upool", bufs=2))
        vpool = ctx.enter_context(tc.tile_pool(name="vpool", bufs=2))
        qpool = ctx.enter_context(tc.tile_pool(name="qpool", bufs=1))
        opool = ctx.enter_context(tc.tile_pool(name="opool", bufs=2))
        ps_d = ctx.enter_context(tc.tile_pool(name="ps_d", bufs=2, space="PSUM"))
        ps_p = ctx.enter_context(tc.tile_pool(name="ps_p", bufs=2, space="PSUM"))
        ps_head = ctx.enter_context(tc.tile_pool(name="ps_head", bufs=2, space="PSUM"))

        # ---- first-needed data first: step-0/ct-0 weights, then features ----
        dg00 = dpool.tile([128, 2 * NPT * 128], F16, tag="diag")
        nc.sync.dma_start(out=dg00[:], in_=diag_d[0, 0])
        fsb = []
        for ct in range(NCT):
            t = fpool.tile([128, FTOT], F16, tag=f"f{ct}")
            fsb.append(t)
        # staged feature DMA on the (otherwise idle) GpSimd DMA queue so the
        # step-0 weight DMAs on the sync queue never wait behind it
        def _fpiece(ct, lo, hi):
            nc.gpsimd.dma_start(
                out=fsb[ct][:].rearrange("p (b x) -> p b x", b=3)[:, :, lo:hi],
                in_=fblk_d[ct].rearrange("p (b x) -> p b x", b=3)[:, :, lo:hi],
            )

        _fpiece(0, 0, 1160)
        # prefetch the rest of step 0's weights before the bulk features
        dg01 = dpool.tile([128, 2 * NPT * 128], F16, tag="diag")
        nc.sync.dma_start(out=dg01[:], in_=diag_d[0, 1])
        ug00 = []
        for dyi in range(3):
            t = dpool.tile([128, 65], F16, tag=f"ug{dyi}")
            nc.sync.dma_start(out=t[:], in_=ug_d[0, 0, dyi])
            ug00.append(t)
        _fpiece(0, 1160, FSPLIT)
        _fpiece(1, 0, FSPLIT)
        _fpiece(0, FSPLIT, BLK)
        _fpiece(1, FSPLIT, BLK)

        # ---- constants ----
        ut_sb = const.tile([64, WO], F16, tag="ut")
        nc.sync.dma_start(out=ut_sb[:], in_=ut_d[:])
        eye_sb = const.tile([128, 128], F16, tag="eye")
        nc.sync.dma_start(out=eye_sb[:], in_=eye_d[:])
        ones3_sb = const.tile([65, 1], F16, tag="ones3")
        nc.sync.dma_start(out=ones3_sb[:], in_=ones3_d[:])
        invw_sb, signw_sb = [], []
        for ct in range(NCT):
            t = const.tile([128, 1], F32, tag=f"invw{ct}")
            nc.sync.dma_start(out=t[:], in_=invw_d[ct])
            invw_sb.append(t)
            t = const.tile([128, 1], F16, tag=f"signw{ct}")
            nc.sync.dma_start(out=t[:], in_=signw_d[ct])
            signw_sb.append(t)

        def emit_head(s, rsp_ct, pq):
            # ---- head: dmap chunks = R2h shifts + sign(w)-weighted t sums ----
            dmY = opool.tile([64, 64], F16, tag="dmY")
            for k in range(HW // 512):
                pd = ps_head.tile([1, 512], F32, tag="hps")
                for j in range(3):
                    base = 32 * j
                    nc.tensor.matmul(
                        pd[:],
                        eye_sb[0:65, base: base + 1],
                        pq[0:65, j + k * 512: j + k * 512 + 512],
                        start=(j == 0),
                        stop=False,
                    )
                for ct in range(NCT):
                    nc.tensor.matmul(
                        pd[:],
                        signw_sb[ct][:],
                        rsp_ct[ct][0][:, k * 512: (k + 1) * 512],
                        start=False,
                        stop=(ct == NCT - 1),
                    )
                dm1 = opool.tile([1, 512], F16, tag="dm1")
                nc.scalar.activation(dm1[:], pd[:], AF.Relu, bias=b_val)
                nc.sync.dma_start(
                    out=dmY[8 * k: 8 * k + 8, :],
                    in_=dm1[:].rearrange("p (y x) -> p y x", x=64),
                )
            # transpose -> x on partitions
            psT0 = ps_head.tile([64, 64], F16, tag="hps")
            nc.tensor.transpose(psT0[:], dmY[:], eye_sb[0:64, 0:64])
            dmX = opool.tile([64, 64], F16, tag="dmX")
            nc.vector.tensor_copy(dmX[:], psT0[:])

            # horizontal upsample in one matmul: H[y, X] (dmX stationary)
            ps_h = ps_head.tile([64, 512], F32, tag="hps")
            nc.tensor.matmul(ps_h[:], dmX[:], ut_sb[:], start=True, stop=True)
            h_sb = opool.tile([64, 512], F16, tag="h_sb")
            nc.scalar.activation(h_sb[:], ps_h[:], AF.Copy)
            # vertical upsample: out[Y, X] = sum_y ut[y, Y] * H[y, X]
            for yc in range(4):
                pv = ps_head.tile([128, 512], F32, tag="hps")
                nc.tensor.matmul(
                    pv[:],
                    ut_sb[:, yc * 128: (yc + 1) * 128],
                    h_sb[:],
                    start=True,
                    stop=True,
                )
                osb = opool.tile([128, 512], F16, tag="osb")
                nc.scalar.activation(osb[:], pv[:], AF.Copy)
                nc.sync.dma_start(out=out_d[s, yc * 128: (yc + 1) * 128, :], in_=osb[:])

        for s in range(STEPS):
            wy = dpool.tile([128, NCT * 2 * 2], F32, tag="wdy")
            nc.sync.dma_start(out=wy[:], in_=wdy_d[s])

            # R2-head stationaries for this step
            ug_sb = []
            for ct in range(NCT):
                if s == 0 and ct == 0:
                    ug_sb.append(ug00)
                else:
                    tiles = []
                    for dyi in range(3):
                        t = dpool.tile([128, 65], F16, tag=f"ug{ct}_{dyi}" if ct else f"ug{dyi}")
                        nc.sync.dma_start(out=t[:], in_=ug_d[s, ct, dyi])
                        tiles.append(t)
                    ug_sb.append(tiles)

            # ---- convs (PE taps + fused DVE taps/merge) + R2h P-pass ----
            rsp_ct = []
            pq = qpool.tile([65, PQW], F16, tag="pq")
            for ct in range(NCT):
                if s == 0 and ct == 0:
                    dg = dg00
                elif s == 0 and ct == 1:
                    dg = dg01
                else:
                    dg = dpool.tile([128, 2 * NPT * 128], F16, tag="diag")
                    nc.sync.dma_start(out=dg[:], in_=diag_d[s, ct])
                dgv = dg[:].rearrange("p (c pt x) -> p c pt x", c=2, pt=NPT)
                rsp = []
                for cv in range(2):
                    r = vpool.tile([128, HW + 32 * cv], F16, tag=f"rsp{ct}{cv}")
                    rsp.append(r)
                rsp_ct.append(rsp)
                for g in range(NGR):
                    gsl = slice(g * GR, (g + 1) * GR)
                    for cv in range(2):
                        acc = ps_d.tile([128, GR], F32, tag="acc")
                        for pi in range(NPT):
                            stat = dgv[:, cv, pi, :]
                            off = PTAP_OFFS[pi] + g * GR
                            for sub in range(GR // 512):
                                nc.tensor.matmul(
                                    acc[:, sub * 512: (sub + 1) * 512],
                                    stat,
                                    fsb[ct][:, off + sub * 512: off + (sub + 1) * 512],
                                    start=(pi == 0),
                                    stop=(pi == NPT - 1),
                                )
                        # fused off-PE taps + psum merge on DVE
                        wix = (ct * 2 + cv) * 2
                        r = rsp[cv]
                        nc.vector.affine_then_add(
                            out=r[:, gsl],
                            in0=fsb[ct][:, OTAP_OFFS[0] + g * GR: OTAP_OFFS[0] + (g + 1) * GR],
                            in1=acc[:],
                            scale=wy[:, wix: wix + 1],
                            bias=0.0,
                        )
                        nc.vector.affine_then_add(
                            out=r[:, gsl],
                            in0=fsb[ct][:, OTAP_OFFS[1] + g * GR: OTAP_OFFS[1] + (g + 1) * GR],
                            in1=r[:, gsl],
                            scale=wy[:, wix + 1: wix + 2],
                            bias=0.0,
                        )
            # P psum chunks: [65, 512] x8, 6 matmuls each (3 dy x 2 ct)
            for k in range(HW // 512):
                pp = ps_p.tile([65, 512], F32, tag="pp")
                first = True
                for ct in range(NCT):
                    for dyi, dy in enumerate((-1, 0, 1)):
                        off = B3 + 64 + 64 * dy + k * 512
                        nc.tensor.matmul(
                            pp[:],
                            ug_sb[ct][dyi][:],
                            fsb[ct][:, off: off + 512],
                            start=first,
                            stop=(ct == NCT - 1 and dyi == 2),
                        )
                        first = False
                nc.scalar.activation(pq[:, 1 + k * 512: 1 + (k + 1) * 512], pp[:], AF.Copy)
            # edge fixes on Pq: zero pads (all rows: avoid NaN*0 from garbage)
            nc.vector.memset(pq[0:65, 0:1], 0.0)
            nc.vector.memset(pq[0:65, PQW - 1: PQW], 0.0)
            pqv0 = pq[0:1, 1: 1 + HW].rearrange("p (y x) -> p y x", x=64)
            nc.vector.memset(pqv0[:, :, 63:64], 0.0)
            pqv2 = pq[64:65, 1: 1 + HW].rearrange("p (y x) -> p y x", x=64)
            nc.vector.memset(pqv2[:, :, 0:1], 0.0)

            # ---- softmax chain (in place, halves) ----
            e_ct = []
            hsls = [slice(0, HW // 2), slice(HW // 2, HW)]
            for ct in range(NCT):
                e0 = upool.tile([128, HW + 32], F16, tag=f"e0{ct}")
                e1 = upool.tile([128, HW + 64], F16, tag=f"e1{ct}")
                e_ct.append((e0, e1))
                for hs in hsls:
                    nc.scalar.activation(e0[:, hs], rsp_ct[ct][0][:, hs], AF.Exp)
                    nc.scalar.activation(e1[:, hs], rsp_ct[ct][1][:, hs], AF.Exp)
            for ct in range(NCT):
                D0r, D1r = rsp_ct[ct]
                e0, e1 = e_ct[ct]
                for hs in hsls:
                    nc.vector.tensor_mul(D0r[:, hs], D0r[:, hs], e0[:, hs])
                    nc.vector.tensor_mul(D1r[:, hs], D1r[:, hs], e1[:, hs])
                    nc.vector.tensor_add(D0r[:, hs], D0r[:, hs], D1r[:, hs])
                    nc.vector.tensor_add(e0[:, hs], e0[:, hs], e1[:, hs])
            for ct in range(NCT):
                e0, e1 = e_ct[ct]
                nc.scalar.activation(e1[:, 0:HW], e0[:, 0:HW], AF.Ln, scale=invw_sb[ct][:, 0:1], bias=invw_sb[ct][:, 0:1])
            for ct in range(NCT):
                e0, e1 = e_ct[ct]
                nc.scalar.activation(e0[:, 0:HW], e1[:, 0:HW], AF.Exp, scale=-1.0)
            for ct in range(NCT):
                D0r, D1r = rsp_ct[ct]
                e0, e1 = e_ct[ct]
                for hs in hsls:
                    nc.vector.tensor_mul(D0r[:, hs], D0r[:, hs], e0[:, hs])

            emit_head(s, rsp_ct, pq)

    nc.compile()
    return nc


_CACHE = {}


def _get_nc(b_val: float) -> bass.Bass:
    key = round(b_val, 12)
    if key not in _CACHE:
        _CACHE[key] = _build_nc(b_val)
    return _CACHE[key]


def kernel(f_e, all_prototypes, w_head, b_head):
    in_maps, b_val = _host_prep(f_e, all_prototypes, w_head, b_head)
    nc = _get_nc(b_val)
    res = run_bass_kernel_spmd(nc, in_maps, list(range(NCORES)), trace=False)
    outs = [res.results[b]["out"].reshape(STEPS, 1, HO, WO) for b in range(BS)]
    full = np.stack(outs, axis=1)  # [STEPS, BS, 1, HO, WO]
    return full.astype(np.float32)


# revision 37
# speedup vs baseline: 1.2740x; 1.0138x over previous
"""LOCA-style kernel for Trainium2, data-parallel over batch on 8 NeuronCores.

Per core (one batch element), per step:
  - depthwise 3x3 correlation for D0=conv(w0-w2), D1=conv(w1-w2): 7 taps
    as fp16 diagonal-weight PE matmuls on a flat-raster feature map
    (zero-edge-column variants give exact padding); the 2 remaining (dy,0)
    taps are fused with the PSUM merge via DVE affine_then_add
    (out = f_shift*wdy + acc), one op per tap per granule.
  - R2's head contribution is linear, so it never materializes per-channel:
    P[dx, n] = sum_dy sum_c (w_head*w2)[c,dy,dx] * f[c, n+64*dy] via 6
    matmul passes with [128 -> {0,32,64}] stationaries, then 3 tiny K=1
    shift-matmuls per 512-chunk accumulate P's dx-shifts straight into the
    head PSUM (edge wrap killed by strided memsets on the SBUF copy).
  - softmax-weighted object sum via the shift identity
      red = R2 + (D0*e0 + D1*e1) / (1 + e0 + e1),  e_i = exp(D_i)
    with exps/Ln on ScalarE (|w_head| folded into the Ln scale/bias),
    products and sums on DVE fp16 2x mode, all in-place on 4 full-map tags.
  - 1x1 head with sign(w_head) stationary + ReLU + 8x bilinear upsample
    as two separable matmul passes. Output fp16, upcast on host.
"""

import sys

sys.path.insert(0, "/opt/trn_rl_repo")

import numpy as np
from contextlib import ExitStack

import concourse.bass as bass
import concourse.mybir as mybir
from concourse import bacc, tile
from concourse.bass_utils import run_bass_kernel_spmd

BS, C, H, W = 8, 256, 64, 64
STEPS, NO = 3, 3
RED = 8
HO, WO = H * RED, W * RED  # 512, 512
NCORES = 8
NCT = 2
HW = H * W  # 4096
GR = 1024  # conv psum granule (pixels)
NGR = HW // GR  # 4
F16 = mybir.dt.float16
F32 = mybir.dt.float32
AF = mybir.ActivationFunctionType
ALU = mybir.AluOpType

# flat f8 layout: one tile [128, 3*BLK] per ct;
#   block 0 (Vz63): col x=63 zeroed, x-origin at 65 (for dx=-1 taps)
#   block 1 (Vz0):  col x=0 zeroed, x-origin at 65 (for dx=+1 taps)
#   block 2 (V0):   full map, x-origin at 64 (for dx=0 taps)
BLK = 4232  # 64 head pad + 4096 + 72 tail pad, even
B1, B2, B3 = 0, BLK, 2 * BLK
FTOT = 3 * BLK
FSPLIT = 2180  # first-piece DMA covers granules 0-1 of every tap


def _tap_off(dy, dx):
    if dx == -1:
        return B1 + 64 + 64 * dy
    if dx == 1:
        return B2 + 66 + 64 * dy
    return B3 + 64 + 64 * dy


# PE taps (x-shifted + center, fp16 diag matmuls); off-PE taps ride DVE
# affine_then_add ops that fuse tap multiply with the psum merge.
PTAPS = [(-1, -1), (-1, 1), (0, -1), (0, 0), (0, 1), (1, -1), (1, 1)]
OTAPS = [(-1, 0), (1, 0)]
NPT = len(PTAPS)
PTAP_OFFS = [_tap_off(*t) for t in PTAPS]
for _o in PTAP_OFFS:
    assert _o % 2 == 0, _o
OTAP_OFFS = [_tap_off(*t) for t in OTAPS]

PQW = 1 + HW + 1  # padded Pq row width


def _bilinear_matrix(n_in: int, n_out: int) -> np.ndarray:
    U = np.zeros((n_out, n_in), np.float64)
    s = n_in / n_out
    for i in range(n_out):
        c = (i + 0.5) * s - 0.5
        lo = int(np.floor(c))
        f = c - lo
        for idx, wt in ((lo, 1.0 - f), (lo + 1, f)):
            U[i, min(max(idx, 0), n_in - 1)] += wt
    return U


def _host_prep(f_e, all_prototypes, w_head, b_head):
    f_e = np.asarray(f_e, np.float32)
    ap = np.asarray(all_prototypes, np.float32)
    w_head = np.asarray(w_head, np.float32)
    b_val = float(np.asarray(b_head).reshape(-1)[0])

    # ---- flat fp16 variants ----
    f16 = f_e.astype(np.float16)  # [BS, C, H, W]
    z63 = f16.copy()
    z63[:, :, :, 63] = 0
    z0 = f16.copy()
    z0[:, :, :, 0] = 0
    fblk = np.zeros((BS, NCT, 128, FTOT), np.float16)
    for ct in range(NCT):
        sl = slice(ct * 128, (ct + 1) * 128)
        fblk[:, ct, :, B1 + 65: B1 + 65 + HW] = z63[:, sl].reshape(BS, 128, HW)
        fblk[:, ct, :, B2 + 65: B2 + 65 + HW] = z0[:, sl].reshape(BS, 128, HW)
        fblk[:, ct, :, B3 + 64: B3 + 64 + HW] = f16[:, sl].reshape(BS, 128, HW)

    # ---- conv weights: D0 = w0-w2, D1 = w1-w2 (fp16) ----
    # ap[s, o*9+t, b, c] -> wm[b, s, o, t(9), c]
    wm = ap.transpose(2, 0, 1, 3).reshape(BS, STEPS, NO, 9, C)
    v = np.stack([wm[:, :, 0] - wm[:, :, 2], wm[:, :, 1] - wm[:, :, 2]], axis=2)
    vf = v.astype(np.float16).astype(np.float32)  # [BS, S, 2, 9, C]

    tapidx = lambda dy, dx: (dy + 1) * 3 + (dx + 1)

    # PE-tap diag stationaries [BS, S, NCT, 2conv, 7tap, 128, 128] fp16
    diag = np.zeros((BS, STEPS, NCT, 2, NPT, 128, 128), np.float16)
    cidx = np.arange(128)
    for ct in range(NCT):
        sl = slice(ct * 128, (ct + 1) * 128)
        for pi, tp in enumerate(PTAPS):
            wa = vf[:, :, :, tapidx(*tp), sl]  # [BS, S, 2, 128]
            diag[:, :, ct, :, pi, cidx, cidx] = wa.transpose(3, 0, 1, 2).astype(np.float16)
    # reorder for per-(step,ct) DMA: -> [BS, S, NCT, 128, 2, 7, 128]
    diag = np.ascontiguousarray(diag.transpose(0, 1, 2, 5, 3, 4, 6))

    # fused-tap scalars for OTAPS: [BS, S, 128, NCT*2conv*3tap] f32
    NOT = len(OTAPS)
    wdy = np.zeros((BS, STEPS, 128, NCT * 2 * NOT), np.float32)
    for ct in range(NCT):
        sl = slice(ct * 128, (ct + 1) * 128)
        for cv in range(2):
            for i, (dy, dx) in enumerate(OTAPS):
                wdy[:, :, :, (ct * 2 + cv) * NOT + i] = vf[:, :, cv, tapidx(dy, dx), sl]

    # R2-head stationaries: u = w_head * w2; ug[b, s, ct, dy, c128, 65] fp16
    # (columns 0/32/64 hold the dx=-1/0/+1 weights so P rows land on
    #  partitions 0/32/64, which are legal K-tile bases for the shift matmuls)
    u = wm[:, :, 2] * w_head[None, None, None, :]  # [BS, S, 9tap, C]
    ug = np.zeros((BS, STEPS, NCT, 3, 128, 65), np.float16)
    for ct in range(NCT):
        sl = slice(ct * 128, (ct + 1) * 128)
        for dyi in range(3):
            for dxi in range(3):
                ug[:, :, ct, dyi, :, 32 * dxi] = u[:, :, dyi * 3 + dxi, sl].astype(np.float16)

    absw = np.abs(w_head).astype(np.float64)
    invw = np.where(absw > 0, 1.0 / np.maximum(absw, 1e-30), 1.0e30)
    invw = np.minimum(invw, 1.0e30).astype(np.float32)
    signw = np.sign(w_head).astype(np.float16)
    invw_t = np.ascontiguousarray(invw.reshape(NCT, 128, 1))
    signw_t = np.ascontiguousarray(signw.reshape(NCT, 128, 1))

    ut = _bilinear_matrix(H, HO).T.astype(np.float16)  # [64, 512]
    eye = np.eye(128, dtype=np.float16)
    ones3 = np.zeros((65, 1), np.float16)
    ones3[[0, 32, 64], 0] = 1.0

    in_maps = []
    for b in range(BS):
        in_maps.append(
            {
                "fblk": np.ascontiguousarray(fblk[b]),
                "diag": np.ascontiguousarray(diag[b]),
                "wdy": np.ascontiguousarray(wdy[b]),
                "ug": np.ascontiguousarray(ug[b]),
                "invw": invw_t,
                "signw": signw_t,
                "ut": ut,
                "eye": eye,
                "ones3": ones3,
            }
        )
    return in_maps, b_val


def _build_nc(b_val: float) -> bass.Bass:
    nc = bacc.Bacc(None, target_bir_lowering=False)
    fblk_d = nc.declare_dram_parameter("fblk", [NCT, 128, FTOT], F16, isOutput=False)
    diag_d = nc.declare_dram_parameter("diag", [STEPS, NCT, 128, 2 * NPT * 128], F16, isOutput=False)
    wdy_d = nc.declare_dram_parameter("wdy", [STEPS, 128, NCT * 2 * 2], F32, isOutput=False)
    ug_d = nc.declare_dram_parameter("ug", [STEPS, NCT, 3, 128, 65], F16, isOutput=False)
    invw_d = nc.declare_dram_parameter("invw", [NCT, 128, 1], F32, isOutput=False)
    signw_d = nc.declare_dram_parameter("signw", [NCT, 128, 1], F16, isOutput=False)
    ut_d = nc.declare_dram_parameter("ut", [64, WO], F16, isOutput=False)
    eye_d = nc.declare_dram_parameter("eye", [128, 128], F16, isOutput=False)
    ones3_d = nc.declare_dram_parameter("ones3", [65, 1], F16, isOutput=False)
    out_d = nc.declare_dram_parameter("out", [STEPS, HO, WO], F16, isOutput=True)

    with tile.TileContext(nc) as tc, ExitStack() as ctx:
        const = ctx.enter_context(tc.tile_pool(name="const", bufs=1))
        fpool = ctx.enter_context(tc.tile_pool(name="fpool", bufs=1))
        dpool = ctx.enter_context(tc.tile_pool(name="dpool", bufs=2))
        upool = ctx.enter_context(tc.tile_pool(name="upool", bufs=2))
        vpool = ctx.enter_context(tc.tile_pool(name="vpool", bufs=2))
        qpool = ctx.enter_context(tc.tile_pool(name="qpool", bufs=1))
        opool = ctx.enter_context(tc.tile_pool(name="opool", bufs=2))
        ps_d = ctx.enter_context(tc.tile_pool(name="ps_d", bufs=2, space="PSUM"))
        ps_p = ctx.enter_context(tc.tile_pool(name="ps_p", bufs=2, space="PSUM"))
        ps_head = ctx.enter_context(tc.tile_pool(name="ps_head", bufs=2, space="PSUM"))

        # ---- first-needed data first: step-0/ct-0 weights, then features ----
        dg00 = dpool.tile([128, 2 * NPT * 128], F16, tag="diag")
        nc.sync.dma_start(out=dg00[:], in_=diag_d[0, 0])
        fsb = []
        for ct in range(NCT):
            t = fpool.tile([128, FTOT], F16, tag=f"f{ct}")
            fsb.append(t)
        # staged feature DMA on the (otherwise idle) GpSimd DMA queue so the
        # step-0 weight DMAs on the sync queue never wait behind it
        def _fpiece(ct, lo, hi):
            nc.gpsimd.dma_start(
                out=fsb[ct][:].rearrange("p (b x) -> p b x", b=3)[:, :, lo:hi],
                in_=fblk_d[ct].rearrange("p (b x) -> p b x", b=3)[:, :, lo:hi],
            )

        _fpiece(0, 0, 1160)
        # prefetch the rest of step 0's weights before the bulk features
        dg01 = dpool.tile([128, 2 * NPT * 128], F16, tag="diag")
        nc.sync.dma_start(out=dg01[:], in_=diag_d[0, 1])
        ug00 = []
        for dyi in range(3):
            t = dpool.tile([128, 65], F16, tag=f"ug{dyi}")
            nc.sync.dma_start(out=t[:], in_=ug_d[0, 0, dyi])
            ug00.append(t)
        _fpiece(0, 1160, FSPLIT)
        _fpiece(1, 0, FSPLIT)
        _fpiece(0, FSPLIT, BLK)
        _fpiece(1, FSPLIT, BLK)

        # ---- constants ----
        ut_sb = const.tile([64, WO], F16, tag="ut")
        nc.sync.dma_start(out=ut_sb[:], in_=ut_d[:])
        eye_sb = const.tile([128, 128], F16, tag="eye")
        nc.sync.dma_start(out=eye_sb[:], in_=eye_d[:])
        ones3_sb = const.tile([65, 1], F16, tag="ones3")
        nc.sync.dma_start(out=ones3_sb[:], in_=ones3_d[:])
        invw_sb, signw_sb = [], []
        for ct in range(NCT):
            t = const.tile([128, 1], F32, tag=f"invw{ct}")
            nc.sync.dma_start(out=t[:], in_=invw_d[ct])
            invw_sb.append(t)
            t = const.tile([128, 1], F16, tag=f"signw{ct}")
            nc.sync.dma_start(out=t[:], in_=signw_d[ct])
            signw_sb.append(t)

        def emit_head(s, rsp_ct, pq):
            # ---- head: dmap chunks = R2h shifts + sign(w)-weighted t sums ----
            dmY = opool.tile([64, 64], F16, tag="dmY")
            for k in range(HW // 512):
                pd = ps_head.tile([1, 512], F32, tag="hps")
                for j in range(3):
                    base = 32 * j
                    nc.tensor.matmul(
                        pd[:],
                        eye_sb[0:65, base: base + 1],
                        pq[0:65, j + k * 512: j + k * 512 + 512],
                        start=(j == 0),
                        stop=False,
                    )
                for ct in range(NCT):
                    nc.tensor.matmul(
                        pd[:],
                        signw_sb[ct][:],
                        rsp_ct[ct][0][:, k * 512: (k + 1) * 512],
                        start=False,
                        stop=(ct == NCT - 1),
                    )
                dm1 = opool.tile([1, 512], F16, tag="dm1")
                nc.scalar.activation(dm1[:], pd[:], AF.Relu, bias=b_val)
                nc.sync.dma_start(
                    out=dmY[8 * k: 8 * k + 8, :],
                    in_=dm1[:].rearrange("p (y x) -> p y x", x=64),
                )
            # transpose -> x on partitions
            psT0 = ps_head.tile([64, 64], F16, tag="hps")
            nc.tensor.transpose(psT0[:], dmY[:], eye_sb[0:64, 0:64])
            dmX = opool.tile([64, 64], F16, tag="dmX")
            nc.vector.tensor_copy(dmX[:], psT0[:])

            # horizontal upsample in one matmul: H[y, X] (dmX stationary)
            ps_h = ps_head.tile([64, 512], F32, tag="hps")
            nc.tensor.matmul(ps_h[:], dmX[:], ut_sb[:], start=True, stop=True)
            h_sb = opool.tile([64, 512], F16, tag="h_sb")
            nc.scalar.activation(h_sb[:], ps_h[:], AF.Copy)
            # vertical upsample: out[Y, X] = sum_y ut[y, Y] * H[y, X]
            for yc in range(4):
                pv = ps_head.tile([128, 512], F32, tag="hps")
                nc.tensor.matmul(
                    pv[:],
                    ut_sb[:, yc * 128: (yc + 1) * 128],
                    h_sb[:],
                    start=True,
                    stop=True,
                )
                osb = opool.tile([128, 512], F16, tag="osb")
                nc.scalar.activation(osb[:], pv[:], AF.Copy)
                nc.sync.dma_start(out=out_d[s, yc * 128: (yc + 1) * 128, :], in_=osb[:])

        for s in range(STEPS):
            wy = dpool.tile([128, NCT * 2 * 2], F32, tag="wdy")
            nc.sync.dma_start(out=wy[:], in_=wdy_d[s])

            # R2-head stationaries for this step
            ug_sb = []
            for ct in range(NCT):
                if s == 0 and ct == 0:
                    ug_sb.append(ug00)
                else:
                    tiles = []
                    for dyi in range(3):
                        t = dpool.tile([128, 65], F16, tag=f"ug{ct}_{dyi}" if ct else f"ug{dyi}")
                        nc.sync.dma_start(out=t[:], in_=ug_d[s, ct, dyi])
                        tiles.append(t)
                    ug_sb.append(tiles)

            # ---- convs (PE taps + fused DVE taps/merge) + R2h P-pass ----
            rsp_ct = []
            pq = qpool.tile([65, PQW], F16, tag="pq")
            for ct in range(NCT):
                if s == 0 and ct == 0:
                    dg = dg00
                elif s == 0 and ct == 1:
                    dg = dg01
                else:
                    dg = dpool.tile([128, 2 * NPT * 128], F16, tag="diag")
                    nc.sync.dma_start(out=dg[:], in_=diag_d[s, ct])
                dgv = dg[:].rearrange("p (c pt x) -> p c pt x", c=2, pt=NPT)
                rsp = []
                for cv in range(2):
                    r = vpool.tile([128, HW + 32 * cv], F16, tag=f"rsp{ct}{cv}")
                    rsp.append(r)
                rsp_ct.append(rsp)
                for g in range(NGR):
                    gsl = slice(g * GR, (g + 1) * GR)
                    for cv in range(2):
                        acc = ps_d.tile([128, GR], F32, tag="acc")
                        for pi in range(NPT):
                            stat = dgv[:, cv, pi, :]
                            off = PTAP_OFFS[pi] + g * GR
                            for sub in range(GR // 512):
                                nc.tensor.matmul(
                                    acc[:, sub * 512: (sub + 1) * 512],
                                    stat,
                                    fsb[ct][:, off + sub * 512: off + (sub + 1) * 512],
                                    start=(pi == 0),
                                    stop=(pi == NPT - 1),
                                )
                        # fused off-PE taps + psum merge on DVE
                        wix = (ct * 2 + cv) * 2
                        r = rsp[cv]
                        nc.vector.affine_then_add(
                            out=r[:, gsl],
                            in0=fsb[ct][:, OTAP_OFFS[0] + g * GR: OTAP_OFFS[0] + (g + 1) * GR],
                            in1=acc[:],
                            scale=wy[:, wix: wix + 1],
                            bias=0.0,
                        )
                        nc.vector.affine_then_add(
                            out=r[:, gsl],
                            in0=fsb[ct][:, OTAP_OFFS[1] + g * GR: OTAP_OFFS[1] + (g + 1) * GR],
                            in1=r[:, gsl],
                            scale=wy[:, wix + 1: wix + 2],
                            bias=0.0,
                        )
            # P psum chunks: [65, 512] x8, 6 matmuls each (3 dy x 2 ct)
            for k in range(HW // 512):
                pp = ps_p.tile([65, 512], F32, tag="pp")
                first = True
                for ct in range(NCT):
                    for dyi, dy in enumerate((-1, 0, 1)):
                        off = B3 + 64 + 64 * dy + k * 512
                        nc.tensor.matmul(
                            pp[:],
                            ug_sb[ct][dyi][:],
                            fsb[ct][:, off: off + 512],
                            start=first,
                            stop=(ct == NCT - 1 and dyi == 2),
                        )
                        first = False
                nc.scalar.activation(pq[:, 1 + k * 512: 1 + (k + 1) * 512], pp[:], AF.Copy)
            # edge fixes on Pq: zero pads (all rows: avoid NaN*0 from garbage)
            nc.vector.memset(pq[0:65, 0:1], 0.0)
            nc.vector.memset(pq[0:65, PQW - 1: PQW], 0.0)
            pqv0 = pq[0:1, 1: 1 + HW].rearrange("p (y x) -> p y x", x=64)
            nc.vector.memset(pqv0[:, :, 63:64], 0.0)
            pqv2 = pq[64:65, 1: 1 + HW].rearrange("p (y x) -> p y x", x=64)
            nc.vector.memset(pqv2[:, :, 0:1], 0.0)

            # ---- softmax chain (in place, halves) ----
            e_ct = []
            hsls = [slice(0, HW // 2), slice(HW // 2, HW)]
            for ct in range(NCT):
                e0 = upool.tile([128, HW + 32], F16, tag=f"e0{ct}")
                e1 = upool.tile([128, HW + 64], F16, tag=f"e1{ct}")
                e_ct.append((e0, e1))
                for hs in hsls:
                    nc.scalar.activation(e0[:, hs], rsp_ct[ct][0][:, hs], AF.Exp)
                    nc.scalar.activation(e1[:, hs], rsp_ct[ct][1][:, hs], AF.Exp)
            for ct in range(NCT):
                D0r, D1r = rsp_ct[ct]
                e0, e1 = e_ct[ct]
                for hs in hsls:
                    nc.vector.tensor_mul(D0r[:, hs], D0r[:, hs], e0[:, hs])
                    nc.vector.tensor_mul(D1r[:, hs], D1r[:, hs], e1[:, hs])
                    nc.vector.tensor_add(D0r[:, hs], D0r[:, hs], D1r[:, hs])
                    nc.vector.tensor_add(e0[:, hs], e0[:, hs], e1[:, hs])
            lnsl = hsls if s == STEPS - 1 else [slice(0, HW)]
            for ct in range(NCT):
                e0, e1 = e_ct[ct]
                for hs in lnsl:
                    nc.scalar.activation(e1[:, hs], e0[:, hs], AF.Ln, scale=invw_sb[ct][:, 0:1], bias=invw_sb[ct][:, 0:1])
            for ct in range(NCT):
                e0, e1 = e_ct[ct]
                for hs in lnsl:
                    nc.scalar.activation(e0[:, hs], e1[:, hs], AF.Exp, scale=-1.0)
            for ct in range(NCT):
                D0r, D1r = rsp_ct[ct]
                e0, e1 = e_ct[ct]
                for hs in hsls:
                    nc.vector.tensor_mul(D0r[:, hs], D0r[:, hs], e0[:, hs])

            emit_head(s, rsp_ct, pq)

    nc.compile()
    return nc


_CACHE = {}


def _get_nc(b_val: float) -> bass.Bass:
    key = round(b_val, 12)
    if key not in _CACHE:
        _CACHE[key] = _build_nc(b_val)
    return _CACHE[key]


def kernel(f_e, all_prototypes, w_head, b_head):
    in_maps, b_val = _host_prep(f_e, all_prototypes, w_head, b_head)
    nc = _get_nc(b_val)
    res = run_bass_kernel_spmd(nc, in_maps, list(range(NCORES)), trace=False)
    outs = [res.results[b]["out"].reshape(STEPS, 1, HO, WO) for b in range(BS)]
    full = np.stack(outs, axis=1)  # [STEPS, BS, 1, HO, WO]
    return full.astype(np.float32)
